# revision 62
# baseline (speedup 1.0000x reference)
"""Trainium2 Bass kernel for nn_Logic_Learning_Model (declarative logic-rule
point-process log-likelihood).

Algorithm (factorized; validated vs reference at ~4e-7 rel err in numpy):
For each sample, all features are masked weighted sums over per-predicate
event arrays evaluated at 512 query times (255 head-event times + 1 big
sentinel + 256 grid points):

  feat0(q) = e^{2(Ck-tq)} * sum_j [t1_j < tq-TOL] * g_j(Ck)
             g_j(Ck) = [s1_j==1] * e^{2(t1_j-Ck)} * What_j
             What_j  = e^{C2-t1_j} * sum_i [t0_i < t1_j-TOL][s0_i==1] e^{t0_i-C2}
  feat1(q) = e^{Ck-tq} * sum_j [t2_j < tq-TOL] * [s2_j==1] e^{t2_j-Ck}
  feat2(q) = e^{Ck-tq} * ( D'(q) - C'(q) ),  D' = sum [t3<=tq] v3,
             C' = sum [t3<tq-TOL] v3,  v3_j = [s3_j==0] e^{t3_j-Ck}
  sh[idx(q)] = sum_j [tq > th_j] * (sh_j - sh_{j-1,wrap}) + sh_255

Ck is a per-query-block shift (C1=38.4 for tq<38.4, C2=76.8 otherwise) to
keep every exponential inside fp32 range.  Masks are exact 0/1 bf16 tiles:
 - mA/mB/mC: ACT-engine Identity activation computes diff = fl(tq-TOL) - t
   via the fast per-partition bias path; DVE/Pool immediate-scalar compares
   produce the 0/1 masks (exact fp32 reference rounding).
 - mD/mE/mwt: PE outer-product diffs from exact bf16 triple-splits
   (hi+mid+lo == fp32 value), compared on Pool/DVE.
Weighted sums run on the PE as bf16 matmuls with Dekker-split (hi+lo)
weight vectors accumulating in fp32 PSUM; all five groups share one PSUM
bank per sample (rows 0-3 A, 32-35 B, 64-67 C', 96-99 D', 100 E).

Sharding: pure data parallel, 32 samples per core on 8 cores; each core
returns 128 per-(sample,query-tile) partial sums; host adds them up.
"""

import numpy as np

import concourse.bass as bass
import concourse.mybir as mybir
from concourse.tile import TileContext

F32 = mybir.dt.float32
BF16 = mybir.dt.bfloat16
I32 = mybir.dt.int32
U8 = mybir.dt.uint8

NCORES = 8
S = 32          # samples per core
E = 256         # events per predicate
EH = 128        # half (one partition tile)
Q = 512         # query count: 255 head + 1 big sentinel + 256 grid
T_MAX = 76.8
RES = 0.3
TOL = 0.1
C1 = 38.4
C2 = 76.8
BIG = 1000.0

AX = mybir.AxisListType
OP = mybir.AluOpType
ACTF = mybir.ActivationFunctionType


def bcast(ap, n=128):
    """0-stride partition broadcast view of a flat DRAM AP."""
    return bass.AP(ap.tensor, ap.offset, [[0, n]] + list(ap.ap))


def build_nc():
    from concourse.bacc import Bacc
    nc = Bacc(None, target_bir_lowering=False)
    times_d = nc.dram_tensor("times", [S, 5, E], F32, kind="ExternalInput")
    states_d = nc.dram_tensor("states", [S, 5, E], I32, kind="ExternalInput")
    base_d = nc.dram_tensor("base", [1], F32, kind="ExternalInput")
    weights_d = nc.dram_tensor("weights", [3], F32, kind="ExternalInput")
    # queries[s] = [th[1:256], BIG, grid];  queriesp = fl(queries - TOL)
    queries_d = nc.dram_tensor("queries", [S, Q], F32, kind="ExternalInput")
    queriesp_d = nc.dram_tensor("queriesp", [S, Q], F32, kind="ExternalInput")
    times1p_d = nc.dram_tensor("times1p", [S, E], F32, kind="ExternalInput")
    onesbf_d = nc.dram_tensor("onesbf", [S * Q], BF16, kind="ExternalInput")
    # consts[:, 0] = qtmask (1 for head rows), consts[:, 1] = pad column mask
    consts_d = nc.dram_tensor("consts", [128, 2], F32, kind="ExternalInput")
    out_d = nc.dram_tensor("out", [128], F32, kind="ExternalOutput")

    with TileContext(nc) as tc:
        _build(tc, nc, times_d, states_d, base_d, weights_d, queries_d,
               queriesp_d, times1p_d, onesbf_d, consts_d, out_d)
    nc.finalize()
    return nc


def _build(tc, nc, times_d, states_d, base_d, weights_d, queries_d,
           queriesp_d, times1p_d, onesbf_d, consts_d, out_d):
    cp = tc.alloc_tile_pool(name="const", bufs=1)
    sp = tc.alloc_tile_pool(name="scr", bufs=3)
    s1 = tc.alloc_tile_pool(name="scr1", bufs=1)
    pp = tc.alloc_tile_pool(name="psum", bufs=1, space="PSUM")

    # ---------------- phase 0: loads + batched prep ----------------
    T = {}
    ST = {}
    for a in range(5):
        for kt in range(2):
            t_t = cp.tile([EH, S], F32, tag=f"T{a}{kt}", name="t")
            src_t = times_d[:, a, kt * EH:(kt + 1) * EH].rearrange("s p -> p s")
            nc.sync.dma_start(out=t_t[:], in_=src_t)
            T[a, kt] = t_t
            s_t = cp.tile([EH, S], I32, tag=f"S{a}{kt}", name="t")
            src_s = states_d[:, a, kt * EH:(kt + 1) * EH].rearrange("s p -> p s")
            nc.sync.dma_start(out=s_t[:], in_=src_s)
            ST[a, kt] = s_t

    # base/weights broadcast columns
    wbbc = cp.tile([128, 4], F32, tag="wbbc", name="t")
    nc.vector.memset(wbbc[:], 0.0)
    nc.sync.dma_start(out=wbbc[:, 0:3], in_=bcast(weights_d[:]))
    nc.sync.dma_start(out=wbbc[:, 3:4], in_=bcast(base_d[:]))
    negw2 = cp.tile([128, 1], F32, tag="negw2", name="t")
    nc.vector.tensor_scalar(out=negw2[:], in0=wbbc[:, 2:3], scalar1=-1.0,
                            scalar2=None, op0=OP.mult)

    consts = cp.tile([128, 2], F32, tag="consts", name="t")
    nc.sync.dma_start(out=consts[:], in_=consts_d[:])
    qtmask = consts[:, 0:1]
    padcol = consts[:, 1:2]

    # sigmoid-mask bias columns: -SCALE*t.  ACT computes
    # sigmoid(SCALE*tq + bias) which saturates to exactly 0.0/1.0 in fp32,
    # fusing diff+compare into one ACT op (threshold noise ~eps*t, negligible).
    SCALE = 1.0e20
    negS = {}
    for a in (0, 1, 2, 3):
        for kt in range(2):
            m = cp.tile([EH, S], F32, tag=f"negS{a}{kt}", name="t")
            nc.vector.tensor_scalar(out=m[:], in0=T[a, kt][:], scalar1=-SCALE,
                                    scalar2=None, op0=OP.mult)
            negS[a, kt] = m

    # batched exponentials / state masks
    ew = {}
    sm = {}
    for kt in range(2):
        def _exp(tag, src, scale, off):
            arg = sp.tile([EH, S], F32, tag=f"arg{tag}{kt}", name="t")
            nc.vector.tensor_scalar(out=arg[:], in0=src[:], scalar1=scale,
                                    scalar2=off, op0=OP.mult, op1=OP.add)
            e_t = cp.tile([EH, S], F32, tag=f"e{tag}{kt}", name="t")
            nc.scalar.activation(e_t[:], arg[:], ACTF.Exp)
            return e_t

        ew["w0", kt] = _exp("w0", T[0, kt], 1.0, -C2)
        ew["c2t1", kt] = _exp("c2t1", T[1, kt], -1.0, C2)
        ew["g1", kt] = _exp("g1", T[1, kt], 2.0, -2.0 * C1)
        ew["g2", kt] = _exp("g2", T[1, kt], 2.0, -2.0 * C2)
        ew["v21", kt] = _exp("v21", T[2, kt], 1.0, -C1)
        ew["v22", kt] = _exp("v22", T[2, kt], 1.0, -C2)
        ew["v31", kt] = _exp("v31", T[3, kt], 1.0, -C1)
        ew["v32", kt] = _exp("v32", T[3, kt], 1.0, -C2)

        for a, val, tag in ((0, 1, "s0"), (1, 1, "s1"), (2, 1, "s2"), (3, 0, "s3")):
            m = cp.tile([EH, S], F32, tag=f"{tag}{kt}", name="t")
            nc.vector.tensor_scalar(out=m[:], in0=ST[a, kt][:], scalar1=val,
                                    scalar2=None, op0=OP.is_equal)
            sm[tag, kt] = m

        # [t3 <= C1]: zero v3C1 entries no C1-block query can select (keeps
        # PSUM partial sums small)
        m31 = cp.tile([EH, S], F32, tag=f"m31{kt}", name="t")
        nc.vector.tensor_scalar(out=m31[:], in0=T[3, kt][:], scalar1=C1,
                                scalar2=None, op0=OP.is_le)
        sm["m31", kt] = m31

    def dekker(dst, blk0, src32, tmp_tag, eng=None):
        """write bf16 (hi, lo) blocks of src32 [128, S] into dst block cols"""
        eng = eng or nc.vector
        hi = dst[:, blk0 * S:(blk0 + 1) * S]
        lo = dst[:, (blk0 + 1) * S:(blk0 + 2) * S]
        eng.tensor_copy(out=hi, in_=src32[:])
        tmp = sp.tile([EH, S], F32, tag=tmp_tag, name="t")
        eng.tensor_copy(out=tmp[:], in_=hi)
        eng.tensor_tensor(out=lo, in0=src32[:], in1=tmp[:], op=OP.subtract)

    # w0 pairs (feat0 inner sum weights)
    w0pair = {}
    for kt in range(2):
        w0 = sp.tile([EH, S], F32, tag=f"w0m{kt}", name="t")
        nc.vector.tensor_tensor(out=w0[:], in0=ew["w0", kt][:], in1=sm["s0", kt][:],
                                op=OP.mult)
        pair = cp.tile([EH, 2 * S], BF16, tag=f"w0pair{kt}", name="t")
        dekker(pair, 0, w0, f"w0tmp{kt}")
        w0pair[kt] = pair

    # v2 / v3 quads [128, 4*S]: cols 4s..4s+3 = [vC1h vC1l vC2h vC2l]
    vB = {}
    vC = {}
    for kt in range(2):
        q_b = cp.tile([EH, 4 * S], BF16, tag=f"vB{kt}", name="t")
        q_c = cp.tile([EH, 4 * S], BF16, tag=f"vC{kt}", name="t")
        for ver, (e2tag, e3tag) in enumerate((("v21", "v31"), ("v22", "v32"))):
            v2 = sp.tile([EH, S], F32, tag=f"v2m{kt}{ver}", name="t")
            nc.vector.tensor_tensor(out=v2[:], in0=ew[e2tag, kt][:],
                                    in1=sm["s2", kt][:], op=OP.mult)
            dekker(q_b, 2 * ver, v2, f"dkb{kt}{ver}")
            v3 = sp.tile([EH, S], F32, tag=f"v3m{kt}{ver}", name="t")
            nc.vector.tensor_tensor(out=v3[:], in0=ew[e3tag, kt][:],
                                    in1=sm["s3", kt][:], op=OP.mult)
            if ver == 0:
                nc.vector.tensor_tensor(out=v3[:], in0=v3[:],
                                        in1=sm["m31", kt][:], op=OP.mult)
            dekker(q_c, 2 * ver, v3, f"dkc{kt}{ver}")
        vB[kt] = q_b
        vC[kt] = q_c

    # dsh (bf16): sh_j - sh_{j-1 (wrap)}; zero-padded [z z z z dsh]
    shm1 = {0: sp.tile([EH, S], I32, tag="shm10", name="t"),
            1: sp.tile([EH, S], I32, tag="shm11", name="t")}
    nc.vector.memset(shm1[0][:], 0)
    nc.vector.memset(shm1[1][:], 0)
    nc.sync.dma_start(out=shm1[0][1:128, :], in_=ST[4, 0][0:127, :])
    nc.sync.dma_start(out=shm1[0][0:1, :], in_=ST[4, 1][127:128, :])
    nc.sync.dma_start(out=shm1[1][1:128, :], in_=ST[4, 1][0:127, :])
    nc.sync.dma_start(out=shm1[1][0:1, :], in_=ST[4, 0][127:128, :])
    dsh = {}
    for kt in range(2):
        d = cp.tile([EH, 5 * S], BF16, tag=f"dsh{kt}", name="t")
        nc.vector.memset(d[:], 0.0)
        nc.vector.tensor_tensor(out=d[:, 4 * S:5 * S], in0=ST[4, kt][:],
                                in1=shm1[kt][:], op=OP.subtract)
        dsh[kt] = d

    # escol = 1 - 2*sh[255], per (sample,qt) partition column
    sh255row = sp.tile([1, S], I32, tag="sh255row", name="t")
    nc.sync.dma_start(out=sh255row[:], in_=ST[4, 1][127:128, :])
    esrow = cp.tile([1, S], F32, tag="esrow", name="t")
    nc.vector.tensor_scalar(out=esrow[:], in0=sh255row[:], scalar1=-2.0,
                            scalar2=1.0, op0=OP.mult, op1=OP.add)
    escol = cp.tile([128, 1], F32, tag="escol", name="t")
    nc.vector.memset(escol[:], 0.0)
    for qt in range(4):
        nc.sync.dma_start(out=escol[32 * qt:32 * (qt + 1), :], in_=esrow[0:1, :])

    # identity for PE transposes
    iot = sp.tile([128, 128], I32, tag="iot", name="t")
    nc.gpsimd.iota(iot[:], pattern=[[1, 128]], base=0, channel_multiplier=-1)
    ident = cp.tile([128, 128], F32, tag="ident", name="t")
    nc.vector.tensor_scalar(out=ident[:], in0=iot[:], scalar1=0,
                            scalar2=None, op0=OP.is_equal)

    # PSUM tiles (8 banks: acc0-3 | pd0-2 | psw); the transpose scratch
    # reuses pd tiles before any diff runs.  Each acc bank holds all four
    # accumulation groups at row offsets 0/32/64/96 (A, B, C', D'+E).
    psw = pp.tile([128, 4], F32, tag="psw", name="t")
    pdw_ps = [pp.tile([128, Q], F32, tag=f"pd{i}", name="t") for i in range(3)]
    acc_ps = [pp.tile([128, Q], F32, tag=f"acc{i}", name="t") for i in range(4)]

    # ---- triple-split helpers ----
    def split3(dst_list, src_ap, part, cols, eng):
        """exact fp32 = hi+mid+lo bf16 split; dst_list = 3 bf16 tiles"""
        hi, mid, lo = dst_list
        r1 = s1.tile([part, cols], F32, tag=f"s3r1_{part}_{cols}", name="t")
        r1f = s1.tile([part, cols], F32, tag=f"s3rf_{part}_{cols}", name="t")
        eng.tensor_copy(out=hi[:], in_=src_ap)
        eng.tensor_copy(out=r1f[:], in_=hi[:])
        eng.tensor_tensor(out=r1[:], in0=src_ap, in1=r1f[:], op=OP.subtract)
        eng.tensor_copy(out=mid[:], in_=r1[:])
        eng.tensor_copy(out=r1f[:], in_=mid[:])
        eng.tensor_tensor(out=r1[:], in0=r1[:], in1=r1f[:], op=OP.subtract)
        eng.tensor_copy(out=lo[:], in_=r1[:])

    # query rows + their splits: qrow [32, Q] (tq)
    qrow = s1.tile([S, Q], F32, tag="qrow", name="t")
    nc.sync.dma_start(out=qrow[:], in_=queries_d[:])

    qspl = [s1.tile([S, Q], BF16, tag=f"qspl{k}", name="t") for k in range(3)]
    split3(qspl, qrow[:], S, Q, nc.vector)

    # negated transposed event splits for the PE-diff stationaries:
    # t3 (mD), t4 (mE)
    ttspl = {}
    for i, (a, kt) in enumerate([(3, 0), (3, 1), (4, 0), (4, 1)]):
        ps = pdw_ps[i % 3]
        nc.tensor.transpose(ps[0:S, 0:128], T[a, kt][:], ident[:])
        tt = s1.tile([S, 128], F32, tag=f"tt{i%2}", name="t")
        nc.scalar.copy(tt[:], ps[0:S, 0:128])
        ntt = s1.tile([S, 128], F32, tag=f"ntt{i%2}", name="t")
        nc.vector.tensor_scalar(out=ntt[:], in0=tt[:], scalar1=-1.0,
                                scalar2=None, op0=OP.mult)
        spl = [s1.tile([S, 128], BF16, tag=f"nts{a}{kt}{k}", name="t")
               for k in range(3)]
        split3(spl, ntt[:], S, 128, nc.vector)
        ttspl[a, kt] = spl

    # 6-partition stacks, free dim = sample-major:
    #   stationary [6, S*ncols]: rows 0-2 = -splits, rows 3-5 = 1
    #   rhs        [6, S*ncols]: rows 0-2 = 1, rows 3-5 = +query splits
    # per-sample operand = [0:6, s*ncols:(s+1)*ncols]  (base partition 0);
    # ones rows come from DRAM (engine memsets can't start at partition 3).
    def stack6(tagbase, ncols, split_src, neg_first):
        t = cp.tile([6, S * ncols], BF16, tag=tagbase, name="t")
        r0 = 0 if neg_first else 3
        o0 = 3 if neg_first else 0
        nc.gpsimd.dma_start(out=t[o0:o0 + 3, :],
                            in_=bcast(onesbf_d[0:S * ncols], 3))
        for k in range(3):
            nc.gpsimd.dma_start(out=t[r0 + k:r0 + k + 1, :],
                                in_=split_src[k][0:S, 0:ncols])
        return t

    statD = {}
    statE = {}
    for kt in range(2):
        statD[kt] = stack6(f"stD{kt}", 128, ttspl[3, kt], True)
        statE[kt] = stack6(f"stE{kt}", 128, ttspl[4, kt], True)
    rhsQ = stack6("rhQ", Q, qspl, False)
    s1.release()

    # per-sample pools allocated after s1's release so they reuse its space
    qp = tc.alloc_tile_pool(name="qbc", bufs=4)
    mp = tc.alloc_tile_pool(name="mask", bufs=4)
    gp = tc.alloc_tile_pool(name="stga", bufs=2)

    def stk(tile, s, ncols):
        return tile[0:6, s * ncols:(s + 1) * ncols]

    # ------------- phase 1: per-sample What (feat0 inner sums) -------------
    wst = cp.tile([128, 4 * S], F32, tag="wst", name="t")
    t1p_tiles = {}
    mwt_store = {}

    def dma_t1pbc(s):
        t = qp.tile([128, E], F32, tag="t1pbc", name="t")
        nc.sync.dma_start(out=t[:], in_=bcast(times1p_d[s, :]))
        t1p_tiles[s] = t

    dma_t1pbc(0)
    dma_t1pbc(1)
    for i in range(S + 1):
        if i < S:
            if i + 2 < S:
                dma_t1pbc(i + 2)
            mwts = []
            for ikt in range(2):
                mwt = mp.tile([128, E], BF16, tag=f"mwt{ikt}", name="t")
                nc.scalar.activation(mwt[:], t1p_tiles[i][:], ACTF.Sigmoid,
                                     bias=negS[0, ikt][:, i:i + 1], scale=SCALE)
                mwts.append(mwt)
            mwt_store[i] = mwts
        s = i - 1
        if 0 <= s:
            mwts = mwt_store.pop(s)
            for jkt in range(2):
                for ikt in range(2):
                    nc.tensor.matmul(psw[:, 2 * jkt:2 * jkt + 2],
                                     mwts[ikt][:, jkt * EH:(jkt + 1) * EH],
                                     w0pair[ikt][:, s::S][:, 0:2],
                                     start=(ikt == 0), stop=(ikt == 1))
            nc.vector.tensor_copy(out=wst[:, s::S][:, 0:4], in_=psw[:])

    # ------------- phase 2: batched g-vector assembly (feat0 weights) ------
    gA = {}
    for kt in range(2):
        wh = sp.tile([EH, S], F32, tag=f"wh{kt}", name="t")
        nc.vector.tensor_tensor(out=wh[:], in0=wst[:, 2 * kt * S:(2 * kt + 1) * S],
                                in1=wst[:, (2 * kt + 1) * S:(2 * kt + 2) * S],
                                op=OP.add)
        nc.vector.tensor_tensor(out=wh[:], in0=wh[:], in1=ew["c2t1", kt][:],
                                op=OP.mult)
        g_t = cp.tile([EH, 4 * S], BF16, tag=f"gA{kt}", name="t")
        for ver, etag in enumerate(("g1", "g2")):
            g32 = sp.tile([EH, S], F32, tag=f"g32{kt}{ver}", name="t")
            nc.vector.tensor_tensor(out=g32[:], in0=ew[etag, kt][:], in1=wh[:],
                                    op=OP.mult)
            nc.vector.tensor_tensor(out=g32[:], in0=g32[:], in1=sm["s1", kt][:],
                                    op=OP.mult)
            dekker(g_t, 2 * ver, g32, f"dkg{kt}{ver}")
        gA[kt] = g_t

    # ------------- phase 3: per-sample masks + weighted sums ---------------
    for t_ps in acc_ps:
        nc.vector.memset(t_ps[:], 0.0)
    stage2 = cp.tile([128, 20 * 128], F32, tag="stage2", name="t")
    pdi = 0

    tqp_tiles = {}
    mask_store = {}

    def dma_tqpbc(s):
        t = qp.tile([128, Q], F32, tag="tqpbc", name="t")
        nc.sync.dma_start(out=t[:], in_=bcast(queriesp_d[s, :]))
        tqp_tiles[s] = t

    def emit_masks(s):
        nonlocal pdi
        tqpbc = tqp_tiles.pop(s)
        masks = {}
        for kt in range(2):
            # mA/mB/mC: one fused ACT sigmoid-mask op each
            for grp, a in (("A", 1), ("B", 2), ("C", 3)):
                m = mp.tile([128, Q], BF16, tag=f"m{grp}{kt}", name="t")
                nc.scalar.activation(m[:], tqpbc[:], ACTF.Sigmoid,
                                     bias=negS[a, kt][:, s:s + 1], scale=SCALE)
                masks[grp, kt] = m
            # mD/mE: PE split-diffs (PSUM) + DVE compares (mE needs the
            # exact-0 self-compare at head queries)
            for grp, stat, op in (("D", statD[kt], OP.is_ge),
                                  ("E", statE[kt], OP.is_gt)):
                pd = pdw_ps[pdi % 3]
                pdi += 1
                nc.tensor.matmul(pd[:, 0:Q], stk(stat, s, 128),
                                 stk(rhsQ, s, Q), start=True, stop=True)
                m = mp.tile([128, Q], BF16, tag=f"m{grp}{kt}", name="t")
                nc.vector.tensor_scalar(out=m[:], in0=pd[:, 0:Q], scalar1=0.0,
                                        scalar2=None, op0=op)
                masks[grp, kt] = m
        mask_store[s] = masks

    def emit_consumers(s):
        # group g accumulates at acc rows 32g..32g+4 (one rotating bank)
        masks = mask_store.pop(s)
        acc = acc_ps[s % 4]
        for kt in range(2):
            st = (kt == 0)
            sp_ = (kt == 1)
            nc.tensor.matmul(acc[0:4, 0:Q], gA[kt][:, s::S][:, 0:4],
                             masks["A", kt][:], start=st, stop=sp_)
            nc.tensor.matmul(acc[32:36, 0:Q], vB[kt][:, s::S][:, 0:4],
                             masks["B", kt][:], start=st, stop=sp_)
            nc.tensor.matmul(acc[64:68, 0:Q], vC[kt][:, s::S][:, 0:4],
                             masks["C", kt][:], start=st, stop=sp_)
            # D'+E share rows 96-100 (E via the zero-padded dsh lhsT)
            nc.tensor.matmul(acc[96:101, 0:Q], dsh[kt][:, s::S][:, 0:5],
                             masks["E", kt][:], start=st, stop=sp_,
                             tile_position=(0, 96))
            nc.tensor.matmul(acc[96:100, 0:Q], vC[kt][:, s::S][:, 0:4],
                             masks["D", kt][:], start=False, stop=False,
                             skip_group_check=True, tile_position=(0, 96))

    def emit_drain(s):
        # drain acc row-group 32g -> stga col-block g (all starts aligned;
        # PSUM reads only on ACT/DVE), then scatter to stage2 rows (32qt+s)
        acc = acc_ps[s % 4]
        stga = gp.tile([5, 4 * Q], F32, tag="stga", name="t")
        nc.scalar.copy(stga[0:5, 0:Q], acc[0:5, 0:Q])
        for g in range(1, 4):
            nc.vector.tensor_copy(out=stga[0:5, g * Q:(g + 1) * Q],
                                  in_=acc[32 * g:32 * g + 5, 0:Q])
        srcv = stga[:].rearrange("p (g x) -> p g x", g=4)
        for qt in range(4):
            row = 32 * qt + s
            dst = stage2[row:row + 1, :].rearrange(
                "one (j g c) -> one j g c", j=5, g=4)
            eng = nc.sync if qt < 1 else nc.gpsimd
            eng.dma_start(out=dst[:, :, :, :],
                          in_=srcv[:, :, qt * 128:(qt + 1) * 128])

    # software pipeline: tqpbc prefetch (+2), masks (lag 0), consumers
    # (lag 1), drain+scatter (lag 2) — keeps ACT/PE/DVE streams decoupled
    dma_tqpbc(0)
    dma_tqpbc(1)
    dma_tqpbc(2)
    for i in range(S + 2):
        if i < S:
            if i + 3 < S:
                dma_tqpbc(i + 3)
            emit_masks(i)
        if 0 <= i - 1 < S:
            emit_consumers(i - 1)
        if 0 <= i - 2 < S:
            emit_drain(i - 2)

    # ------------- phase 4: batched post-processing ------------------------
    def R(r):
        return stage2[:, r * 128:(r + 1) * 128]

    # query matrix [128 (qt,s), 128]: tq_m[32qt+s, c] = queries[s, 128qt+c]
    tq_m = cp.tile([128, 128], F32, tag="tqm", name="t")
    for qt in range(4):
        nc.sync.dma_start(out=tq_m[32 * qt:32 * (qt + 1), :],
                          in_=queries_d[:, 128 * qt:128 * (qt + 1)])

    def tmp(tag):
        return cp.tile([128, 128], F32, tag=tag, name="t")

    # role r = j*4 + g (j = quad row [C1h C1l C2h C2l], g = group A B C' D');
    # j=4: roles 16-18 junk-zero, 19 = E
    for r in (0, 1, 2, 3, 8, 9, 10, 11):
        nc.vector.tensor_tensor(out=R(r), in0=R(r), in1=R(r + 4), op=OP.add)
    A1, B1, Cs1, Dr1, A2, B2, Cs2, Dr2 = (R(r) for r in (0, 1, 2, 3, 8, 9, 10, 11))

    blk = cp.tile([128, 128], U8, tag="blk", name="t")
    nc.vector.tensor_scalar(out=blk[:], in0=tq_m[:], scalar1=C1, scalar2=None,
                            op0=OP.is_ge)
    biasC1 = cp.tile([128, 1], F32, tag="biasC1", name="t")
    nc.vector.memset(biasC1[:], C1)
    biasC2 = cp.tile([128, 1], F32, tag="biasC2", name="t")
    nc.vector.memset(biasC2[:], C2)
    e1 = tmp("e1")
    nc.scalar.activation(e1[:], tq_m[:], ACTF.Exp, bias=biasC1[:], scale=-1.0)
    e2 = tmp("e2")
    nc.scalar.activation(e2[:], tq_m[:], ACTF.Exp, bias=biasC2[:], scale=-1.0)

    def sel(tag, on_true, on_false):
        o = tmp(tag)
        nc.vector.select(o, blk[:], on_true, on_false)
        return o

    esel = sel("esel", e2[:], e1[:])
    Asel = sel("Asel", A2, A1)
    Bsel = sel("Bsel", B2, B1)
    Csel = sel("Csel", Cs2, Cs1)
    Dsel = sel("Dsel", Dr2, Dr1)

    feat0 = tmp("feat0")
    nc.vector.tensor_tensor(out=feat0[:], in0=esel[:], in1=Asel[:], op=OP.mult)
    nc.vector.tensor_tensor(out=feat0[:], in0=feat0[:], in1=esel[:], op=OP.mult)
    feat1 = tmp("feat1")
    nc.vector.tensor_tensor(out=feat1[:], in0=esel[:], in1=Bsel[:], op=OP.mult)
    feat2 = tmp("feat2")
    nc.vector.tensor_tensor(out=feat2[:], in0=Dsel[:], in1=Csel[:], op=OP.subtract)
    nc.vector.tensor_tensor(out=feat2[:], in0=feat2[:], in1=esel[:], op=OP.mult)

    eff0 = tmp("eff0")
    nc.vector.tensor_scalar(out=eff0[:], in0=R(19), scalar1=-2.0,
                            scalar2=escol[:], op0=OP.mult, op1=OP.add)

    combo = tmp("combo")
    nc.vector.tensor_scalar(out=combo[:], in0=feat0[:], scalar1=wbbc[:, 0:1],
                            scalar2=None, op0=OP.mult)
    nc.vector.scalar_tensor_tensor(out=combo[:], in0=feat1[:], scalar=wbbc[:, 1:2],
                                   in1=combo[:], op0=OP.mult, op1=OP.add)
    nc.vector.scalar_tensor_tensor(out=combo[:], in0=feat2[:], scalar=negw2[:],
                                   in1=combo[:], op0=OP.mult, op1=OP.add)
    logits = tmp("logits")
    nc.vector.tensor_tensor(out=logits[:], in0=combo[:], in1=eff0[:], op=OP.mult)
    nc.vector.tensor_scalar(out=logits[:], in0=logits[:], scalar1=wbbc[:, 3:4],
                            scalar2=None, op0=OP.add)
    # zero the sentinel query (qt==1 rows, col 127)
    nc.vector.tensor_tensor(out=logits[:, 127:128], in0=logits[:, 127:128],
                            in1=padcol, op=OP.mult)

    hsum = cp.tile([128, 1], F32, tag="hsum", name="t")
    nc.vector.tensor_reduce(out=hsum[:], in_=logits[:], axis=AX.X, op=OP.add)
    expt = tmp("expt")
    intcol = cp.tile([128, 1], F32, tag="intcol", name="t")
    nc.scalar.activation(expt[:], logits[:], ACTF.Exp, accum_out=intcol[:])
    nc.vector.tensor_scalar(out=intcol[:], in0=intcol[:], scalar1=-RES,
                            scalar2=None, op0=OP.mult)
    qtmaski = cp.tile([128, 1], U8, tag="qtmaski", name="t")
    nc.vector.tensor_scalar(out=qtmaski[:], in0=qtmask, scalar1=0.5,
                            scalar2=None, op0=OP.is_ge)
    rowpart = cp.tile([128, 1], F32, tag="rowpart", name="t")
    nc.vector.select(rowpart[:], qtmaski[:], hsum[:], intcol[:])
    nc.sync.dma_start(out=out_d[:], in_=rowpart[:])

    for pool in (pp, gp, mp, qp, sp, cp):
        pool.release()


_NC_CACHE = []


def _get_nc():
    if not _NC_CACHE:
        _NC_CACHE.append(build_nc())
    return _NC_CACHE[0]


def make_inputs_for_core(times, states, base, weights, core):
    grid = np.arange(0.0, T_MAX, RES, dtype=np.float32)
    consts = np.ones((128, 2), np.float32)
    consts[64:128, 0] = 0.0   # qtmask: 0 for grid rows (qt 2,3 blocks)
    consts[32:64, 1] = 0.0    # sentinel-column mask: 0 for qt1 block
    sl = slice(core * S, (core + 1) * S)
    t = np.ascontiguousarray(times[sl]).astype(np.float32)
    st = np.ascontiguousarray(states[sl]).astype(np.int32)
    queries = np.concatenate(
        [t[:, 4, 1:256], np.full((S, 1), BIG, np.float32),
         np.tile(grid, (S, 1))], axis=1).astype(np.float32)
    queriesp = (queries - np.float32(TOL)).astype(np.float32)
    times1p = (t[:, 1, :] - np.float32(TOL)).astype(np.float32)
    return {
        "times": t,
        "states": st,
        "base": np.asarray(base, np.float32),
        "weights": np.asarray(weights, np.float32),
        "queries": queries,
        "queriesp": queriesp,
        "times1p": times1p,
        "onesbf": np.ones(S * Q, mybir.dt.np(BF16)),
        "consts": consts,
    }


def kernel(times, states, base, weights):
    from concourse.bass_utils import run_bass_kernel_spmd

    times = np.asarray(times, np.float32)
    states = np.asarray(states, np.int32)
    nc = _get_nc()
    in_maps = [make_inputs_for_core(times, states, base, weights, c)
               for c in range(NCORES)]
    res = run_bass_kernel_spmd(nc, in_maps, list(range(NCORES)))
    parts = np.stack([np.asarray(res.results[c]["out"]) for c in range(NCORES)])
    total = np.sum(parts.astype(np.float32), dtype=np.float32)
    return np.array([total], dtype=np.float32)


def run_traced(times, states, base, weights):
    """Profiled run; returns BassKernelResults (exec_time_ns etc.)."""
    from concourse.bass_utils import run_bass_kernel_spmd

    times = np.asarray(times, np.float32)
    states = np.asarray(states, np.int32)
    nc = _get_nc()
    in_maps = [make_inputs_for_core(times, states, base, weights, c)
               for c in range(NCORES)]
    res = run_bass_kernel_spmd(nc, in_maps, list(range(NCORES)), trace=True)
    return res


# revision 64
# speedup vs baseline: 1.1405x; 1.1405x over previous
"""Trainium2 Bass kernel for nn_Logic_Learning_Model (declarative logic-rule
point-process log-likelihood).

Algorithm (factorized; validated vs reference at ~4e-7 rel err in numpy):
For each sample, all features are masked weighted sums over per-predicate
event arrays evaluated at 512 query times (255 head-event times + 1 big
sentinel + 256 grid points):

  feat0(q) = e^{2(Ck-tq)} * sum_j [t1_j < tq-TOL] * g_j(Ck)
             g_j(Ck) = [s1_j==1] * e^{2(t1_j-Ck)} * What_j
             What_j  = e^{C2-t1_j} * sum_i [t0_i < t1_j-TOL][s0_i==1] e^{t0_i-C2}
  feat1(q) = e^{Ck-tq} * sum_j [t2_j < tq-TOL] * [s2_j==1] e^{t2_j-Ck}
  feat2(q) = e^{Ck-tq} * ( D'(q) - C'(q) ),  D' = sum [t3<=tq] v3,
             C' = sum [t3<tq-TOL] v3,  v3_j = [s3_j==0] e^{t3_j-Ck}
  sh[idx(q)] = sum_j [tq > th_j] * (sh_j - sh_{j-1,wrap}) + sh_255

Ck is a per-query-block shift (C1=38.4 for tq<38.4, C2=76.8 otherwise) to
keep every exponential inside fp32 range.  Masks are exact 0/1 bf16 tiles:
 - mA/mB/mC: ACT-engine Identity activation computes diff = fl(tq-TOL) - t
   via the fast per-partition bias path; DVE/Pool immediate-scalar compares
   produce the 0/1 masks (exact fp32 reference rounding).
 - mD/mE/mwt: PE outer-product diffs from exact bf16 triple-splits
   (hi+mid+lo == fp32 value), compared on Pool/DVE.
Weighted sums run on the PE as bf16 matmuls with Dekker-split (hi+lo)
weight vectors accumulating in fp32 PSUM; all five groups share one PSUM
bank per sample (rows 0-3 A, 32-35 B, 64-67 C', 96-99 D', 100 E).

Sharding: pure data parallel, 32 samples per core on 8 cores; each core
returns 128 per-(sample,query-tile) partial sums; host adds them up.
"""

import numpy as np

import concourse.bass as bass
import concourse.mybir as mybir
from concourse.tile import TileContext

F32 = mybir.dt.float32
BF16 = mybir.dt.bfloat16
I32 = mybir.dt.int32
U8 = mybir.dt.uint8

NCORES = 8
S = 32          # samples per core
E = 256         # events per predicate
EH = 128        # half (one partition tile)
Q = 512         # query count: 255 head + 1 big sentinel + 256 grid
T_MAX = 76.8
RES = 0.3
TOL = 0.1
C1 = 38.4
C2 = 76.8
BIG = 1000.0

AX = mybir.AxisListType
OP = mybir.AluOpType
ACTF = mybir.ActivationFunctionType


def bcast(ap, n=128):
    """0-stride partition broadcast view of a flat DRAM AP."""
    return bass.AP(ap.tensor, ap.offset, [[0, n]] + list(ap.ap))


def build_nc():
    from concourse.bacc import Bacc
    nc = Bacc(None, target_bir_lowering=False)
    timesT_d = nc.dram_tensor("timesT", [5, 2, EH, S], F32, kind="ExternalInput")
    statesT_d = nc.dram_tensor("statesT", [5, 2, EH, S], I32, kind="ExternalInput")
    base_d = nc.dram_tensor("base", [1], F32, kind="ExternalInput")
    weights_d = nc.dram_tensor("weights", [3], F32, kind="ExternalInput")
    # queries[s] = [th[1:256], BIG, grid];  queriesp = fl(queries - TOL)
    queries_d = nc.dram_tensor("queries", [S, Q], F32, kind="ExternalInput")
    queriesp_d = nc.dram_tensor("queriesp", [S, Q], F32, kind="ExternalInput")
    times1p_d = nc.dram_tensor("times1p", [S, E], F32, kind="ExternalInput")
    onesbf_d = nc.dram_tensor("onesbf", [S * Q], BF16, kind="ExternalInput")
    # consts[:, 0] = qtmask (1 for head rows), consts[:, 1] = pad column mask
    consts_d = nc.dram_tensor("consts", [128, 2], F32, kind="ExternalInput")
    out_d = nc.dram_tensor("out", [128], F32, kind="ExternalOutput")

    with TileContext(nc) as tc:
        _build(tc, nc, timesT_d, statesT_d, base_d, weights_d, queries_d,
               queriesp_d, times1p_d, onesbf_d, consts_d, out_d)
    nc.finalize()
    return nc


def _build(tc, nc, timesT_d, statesT_d, base_d, weights_d, queries_d,
           queriesp_d, times1p_d, onesbf_d, consts_d, out_d):
    cp = tc.alloc_tile_pool(name="const", bufs=1)
    sp = tc.alloc_tile_pool(name="scr", bufs=3)
    s1 = tc.alloc_tile_pool(name="scr1", bufs=1)
    pp = tc.alloc_tile_pool(name="psum", bufs=1, space="PSUM")

    # ---------------- phase 0: loads + batched prep ----------------
    T = {}
    ST = {}
    for a in range(5):
        for kt in range(2):
            t_t = cp.tile([EH, S], F32, tag=f"T{a}{kt}", name="t")
            nc.sync.dma_start(out=t_t[:], in_=timesT_d[a, kt])
            T[a, kt] = t_t
            s_t = cp.tile([EH, S], I32, tag=f"S{a}{kt}", name="t")
            nc.sync.dma_start(out=s_t[:], in_=statesT_d[a, kt])
            ST[a, kt] = s_t

    # base/weights broadcast columns
    wbbc = cp.tile([128, 4], F32, tag="wbbc", name="t")
    nc.vector.memset(wbbc[:], 0.0)
    nc.sync.dma_start(out=wbbc[:, 0:3], in_=bcast(weights_d[:]))
    nc.sync.dma_start(out=wbbc[:, 3:4], in_=bcast(base_d[:]))
    negw2 = cp.tile([128, 1], F32, tag="negw2", name="t")
    nc.vector.tensor_scalar(out=negw2[:], in0=wbbc[:, 2:3], scalar1=-1.0,
                            scalar2=None, op0=OP.mult)

    consts = cp.tile([128, 2], F32, tag="consts", name="t")
    nc.sync.dma_start(out=consts[:], in_=consts_d[:])
    qtmask = consts[:, 0:1]
    padcol = consts[:, 1:2]

    # sigmoid-mask bias columns: -SCALE*t.  ACT computes
    # sigmoid(SCALE*tq + bias) which saturates to exactly 0.0/1.0 in fp32,
    # fusing diff+compare into one ACT op (threshold noise ~eps*t, negligible).
    SCALE = 1.0e20
    negS = {}
    for a in (0, 1, 2, 3):
        for kt in range(2):
            m = cp.tile([EH, S], F32, tag=f"negS{a}{kt}", name="t")
            nc.vector.tensor_scalar(out=m[:], in0=T[a, kt][:], scalar1=-SCALE,
                                    scalar2=None, op0=OP.mult)
            negS[a, kt] = m

    # batched exponentials / state masks
    ew = {}
    sm = {}
    for kt in range(2):
        def _exp(tag, src, scale, off):
            arg = sp.tile([EH, S], F32, tag=f"arg{tag}{kt}", name="t")
            nc.vector.tensor_scalar(out=arg[:], in0=src[:], scalar1=scale,
                                    scalar2=off, op0=OP.mult, op1=OP.add)
            e_t = cp.tile([EH, S], F32, tag=f"e{tag}{kt}", name="t")
            nc.scalar.activation(e_t[:], arg[:], ACTF.Exp)
            return e_t

        ew["w0", kt] = _exp("w0", T[0, kt], 1.0, -C2)
        ew["c2t1", kt] = _exp("c2t1", T[1, kt], -1.0, C2)
        ew["g1", kt] = _exp("g1", T[1, kt], 2.0, -2.0 * C1)
        ew["g2", kt] = _exp("g2", T[1, kt], 2.0, -2.0 * C2)
        ew["v21", kt] = _exp("v21", T[2, kt], 1.0, -C1)
        ew["v22", kt] = _exp("v22", T[2, kt], 1.0, -C2)
        ew["v31", kt] = _exp("v31", T[3, kt], 1.0, -C1)
        ew["v32", kt] = _exp("v32", T[3, kt], 1.0, -C2)

        for a, val, tag in ((0, 1, "s0"), (1, 1, "s1"), (2, 1, "s2"), (3, 0, "s3")):
            m = cp.tile([EH, S], F32, tag=f"{tag}{kt}", name="t")
            nc.vector.tensor_scalar(out=m[:], in0=ST[a, kt][:], scalar1=val,
                                    scalar2=None, op0=OP.is_equal)
            sm[tag, kt] = m

        # [t3 <= C1]: zero v3C1 entries no C1-block query can select (keeps
        # PSUM partial sums small)
        m31 = cp.tile([EH, S], F32, tag=f"m31{kt}", name="t")
        nc.vector.tensor_scalar(out=m31[:], in0=T[3, kt][:], scalar1=C1,
                                scalar2=None, op0=OP.is_le)
        sm["m31", kt] = m31

    def dekker(dst, blk0, src32, tmp_tag, eng=None):
        """write bf16 (hi, lo) blocks of src32 [128, S] into dst block cols"""
        eng = eng or nc.vector
        hi = dst[:, blk0 * S:(blk0 + 1) * S]
        lo = dst[:, (blk0 + 1) * S:(blk0 + 2) * S]
        eng.tensor_copy(out=hi, in_=src32[:])
        tmp = sp.tile([EH, S], F32, tag=tmp_tag, name="t")
        eng.tensor_copy(out=tmp[:], in_=hi)
        eng.tensor_tensor(out=lo, in0=src32[:], in1=tmp[:], op=OP.subtract)

    # w0 pairs (feat0 inner sum weights)
    w0pair = {}
    for kt in range(2):
        w0 = sp.tile([EH, S], F32, tag=f"w0m{kt}", name="t")
        nc.vector.tensor_tensor(out=w0[:], in0=ew["w0", kt][:], in1=sm["s0", kt][:],
                                op=OP.mult)
        pair = cp.tile([EH, 2 * S], BF16, tag=f"w0pair{kt}", name="t")
        dekker(pair, 0, w0, f"w0tmp{kt}")
        w0pair[kt] = pair

    # v2 / v3 quads [128, 4*S]: cols 4s..4s+3 = [vC1h vC1l vC2h vC2l]
    vB = {}
    vC = {}
    for kt in range(2):
        q_b = cp.tile([EH, 4 * S], BF16, tag=f"vB{kt}", name="t")
        q_c = cp.tile([EH, 4 * S], BF16, tag=f"vC{kt}", name="t")
        for ver, (e2tag, e3tag) in enumerate((("v21", "v31"), ("v22", "v32"))):
            v2 = sp.tile([EH, S], F32, tag=f"v2m{kt}{ver}", name="t")
            nc.vector.tensor_tensor(out=v2[:], in0=ew[e2tag, kt][:],
                                    in1=sm["s2", kt][:], op=OP.mult)
            dekker(q_b, 2 * ver, v2, f"dkb{kt}{ver}")
            v3 = sp.tile([EH, S], F32, tag=f"v3m{kt}{ver}", name="t")
            nc.vector.tensor_tensor(out=v3[:], in0=ew[e3tag, kt][:],
                                    in1=sm["s3", kt][:], op=OP.mult)
            if ver == 0:
                nc.vector.tensor_tensor(out=v3[:], in0=v3[:],
                                        in1=sm["m31", kt][:], op=OP.mult)
            dekker(q_c, 2 * ver, v3, f"dkc{kt}{ver}")
        vB[kt] = q_b
        vC[kt] = q_c

    # dsh (bf16): sh_j - sh_{j-1 (wrap)}; zero-padded [z z z z dsh]
    shm1 = {0: sp.tile([EH, S], I32, tag="shm10", name="t"),
            1: sp.tile([EH, S], I32, tag="shm11", name="t")}
    nc.vector.memset(shm1[0][:], 0)
    nc.vector.memset(shm1[1][:], 0)
    nc.sync.dma_start(out=shm1[0][1:128, :], in_=ST[4, 0][0:127, :])
    nc.sync.dma_start(out=shm1[0][0:1, :], in_=ST[4, 1][127:128, :])
    nc.sync.dma_start(out=shm1[1][1:128, :], in_=ST[4, 1][0:127, :])
    nc.sync.dma_start(out=shm1[1][0:1, :], in_=ST[4, 0][127:128, :])
    dsh = {}
    for kt in range(2):
        d = cp.tile([EH, 5 * S], BF16, tag=f"dsh{kt}", name="t")
        nc.vector.memset(d[:], 0.0)
        nc.vector.tensor_tensor(out=d[:, 4 * S:5 * S], in0=ST[4, kt][:],
                                in1=shm1[kt][:], op=OP.subtract)
        dsh[kt] = d

    # escol = 1 - 2*sh[255], per (sample,qt) partition column
    sh255row = sp.tile([1, S], I32, tag="sh255row", name="t")
    nc.sync.dma_start(out=sh255row[:], in_=ST[4, 1][127:128, :])
    esrow = cp.tile([1, S], F32, tag="esrow", name="t")
    nc.vector.tensor_scalar(out=esrow[:], in0=sh255row[:], scalar1=-2.0,
                            scalar2=1.0, op0=OP.mult, op1=OP.add)
    escol = cp.tile([128, 1], F32, tag="escol", name="t")
    nc.vector.memset(escol[:], 0.0)
    for qt in range(4):
        nc.sync.dma_start(out=escol[32 * qt:32 * (qt + 1), :], in_=esrow[0:1, :])

    # identity for PE transposes
    iot = sp.tile([128, 128], I32, tag="iot", name="t")
    nc.gpsimd.iota(iot[:], pattern=[[1, 128]], base=0, channel_multiplier=-1)
    ident = cp.tile([128, 128], F32, tag="ident", name="t")
    nc.vector.tensor_scalar(out=ident[:], in0=iot[:], scalar1=0,
                            scalar2=None, op0=OP.is_equal)

    # PSUM tiles (8 banks: acc0-3 | pd0-2 | psw); the transpose scratch
    # reuses pd tiles before any diff runs.  Each acc bank holds all four
    # accumulation groups at row offsets 0/32/64/96 (A, B, C', D'+E).
    psw = pp.tile([128, 4], F32, tag="psw", name="t")
    pdw_ps = [pp.tile([128, Q], F32, tag=f"pd{i}", name="t") for i in range(3)]
    acc_ps = [pp.tile([128, Q], F32, tag=f"acc{i}", name="t") for i in range(4)]

    # ---- triple-split helpers ----
    def split3(dst_list, src_ap, part, cols, eng):
        """exact fp32 = hi+mid+lo bf16 split; dst_list = 3 bf16 tiles"""
        hi, mid, lo = dst_list
        r1 = s1.tile([part, cols], F32, tag=f"s3r1_{part}_{cols}", name="t")
        r1f = s1.tile([part, cols], F32, tag=f"s3rf_{part}_{cols}", name="t")
        eng.tensor_copy(out=hi[:], in_=src_ap)
        eng.tensor_copy(out=r1f[:], in_=hi[:])
        eng.tensor_tensor(out=r1[:], in0=src_ap, in1=r1f[:], op=OP.subtract)
        eng.tensor_copy(out=mid[:], in_=r1[:])
        eng.tensor_copy(out=r1f[:], in_=mid[:])
        eng.tensor_tensor(out=r1[:], in0=r1[:], in1=r1f[:], op=OP.subtract)
        eng.tensor_copy(out=lo[:], in_=r1[:])

    # query rows + their splits: qrow [32, Q] (tq)
    qrow = s1.tile([S, Q], F32, tag="qrow", name="t")
    nc.sync.dma_start(out=qrow[:], in_=queries_d[:])

    qspl = [s1.tile([S, Q], BF16, tag=f"qspl{k}", name="t") for k in range(3)]
    split3(qspl, qrow[:], S, Q, nc.vector)

    # negated transposed event splits for the PE-diff stationaries:
    # t2+TOL (mB), t3 (mD), t4 (mE)
    ttspl = {}
    for i, (a, kt) in enumerate([(2, 0), (2, 1),
                                 (3, 0), (3, 1), (4, 0), (4, 1)]):
        ps = pdw_ps[i % 3]
        nc.tensor.transpose(ps[0:S, 0:128], T[a, kt][:], ident[:])
        tt = s1.tile([S, 128], F32, tag=f"tt{i%2}", name="t")
        nc.scalar.copy(tt[:], ps[0:S, 0:128])
        ntt = s1.tile([S, 128], F32, tag=f"ntt{i%2}", name="t")
        # mB compares against tq (not tq-TOL): fold TOL into the t2 side
        off = -TOL if a == 2 else 0.0
        nc.vector.tensor_scalar(out=ntt[:], in0=tt[:], scalar1=-1.0,
                                scalar2=off, op0=OP.mult, op1=OP.add)
        spl = [s1.tile([S, 128], BF16, tag=f"nts{a}{kt}{k}", name="t")
               for k in range(3)]
        split3(spl, ntt[:], S, 128, nc.vector)
        ttspl[a, kt] = spl

    # 6-partition stacks, free dim = sample-major:
    #   stationary [6, S*ncols]: rows 0-2 = -splits, rows 3-5 = 1
    #   rhs        [6, S*ncols]: rows 0-2 = 1, rows 3-5 = +query splits
    # per-sample operand = [0:6, s*ncols:(s+1)*ncols]  (base partition 0);
    # ones rows come from DRAM (engine memsets can't start at partition 3).
    def stack6(tagbase, ncols, split_src, neg_first):
        t = cp.tile([6, S * ncols], BF16, tag=tagbase, name="t")
        r0 = 0 if neg_first else 3
        o0 = 3 if neg_first else 0
        nc.gpsimd.dma_start(out=t[o0:o0 + 3, :],
                            in_=bcast(onesbf_d[0:S * ncols], 3))
        for k in range(3):
            nc.gpsimd.dma_start(out=t[r0 + k:r0 + k + 1, :],
                                in_=split_src[k][0:S, 0:ncols])
        return t

    statB = {}
    statD = {}
    statE = {}
    for kt in range(2):
        statB[kt] = stack6(f"stB{kt}", 128, ttspl[2, kt], True)
        statD[kt] = stack6(f"stD{kt}", 128, ttspl[3, kt], True)
        statE[kt] = stack6(f"stE{kt}", 128, ttspl[4, kt], True)
    rhsQ = stack6("rhQ", Q, qspl, False)
    s1.release()

    # per-sample pools allocated after s1's release so they reuse its space
    qp = tc.alloc_tile_pool(name="qbc", bufs=4)
    mp = tc.alloc_tile_pool(name="mask", bufs=4)
    gp = tc.alloc_tile_pool(name="stga", bufs=2)

    def stk(tile, s, ncols):
        return tile[0:6, s * ncols:(s + 1) * ncols]

    # ------------- phase 1: per-sample What (feat0 inner sums) -------------
    wst = cp.tile([128, 4 * S], F32, tag="wst", name="t")
    t1p_tiles = {}
    mwt_store = {}

    def dma_t1pbc(s):
        t = qp.tile([128, E], F32, tag="t1pbc", name="t")
        nc.sync.dma_start(out=t[:], in_=bcast(times1p_d[s, :]))
        t1p_tiles[s] = t

    dma_t1pbc(0)
    dma_t1pbc(1)
    for i in range(S + 1):
        if i < S:
            if i + 2 < S:
                dma_t1pbc(i + 2)
            mwts = []
            for ikt in range(2):
                mwt = mp.tile([128, E], BF16, tag=f"mwt{ikt}", name="t")
                nc.scalar.activation(mwt[:], t1p_tiles[i][:], ACTF.Sigmoid,
                                     bias=negS[0, ikt][:, i:i + 1], scale=SCALE)
                mwts.append(mwt)
            mwt_store[i] = mwts
        s = i - 1
        if 0 <= s:
            mwts = mwt_store.pop(s)
            for jkt in range(2):
                for ikt in range(2):
                    nc.tensor.matmul(psw[:, 2 * jkt:2 * jkt + 2],
                                     mwts[ikt][:, jkt * EH:(jkt + 1) * EH],
                                     w0pair[ikt][:, s::S][:, 0:2],
                                     start=(ikt == 0), stop=(ikt == 1))
            nc.vector.tensor_copy(out=wst[:, s::S][:, 0:4], in_=psw[:])

    # ------------- phase 2: batched g-vector assembly (feat0 weights) ------
    gA = {}
    for kt in range(2):
        wh = sp.tile([EH, S], F32, tag=f"wh{kt}", name="t")
        nc.vector.tensor_tensor(out=wh[:], in0=wst[:, 2 * kt * S:(2 * kt + 1) * S],
                                in1=wst[:, (2 * kt + 1) * S:(2 * kt + 2) * S],
                                op=OP.add)
        nc.vector.tensor_tensor(out=wh[:], in0=wh[:], in1=ew["c2t1", kt][:],
                                op=OP.mult)
        g_t = cp.tile([EH, 4 * S], BF16, tag=f"gA{kt}", name="t")
        for ver, etag in enumerate(("g1", "g2")):
            g32 = sp.tile([EH, S], F32, tag=f"g32{kt}{ver}", name="t")
            nc.vector.tensor_tensor(out=g32[:], in0=ew[etag, kt][:], in1=wh[:],
                                    op=OP.mult)
            nc.vector.tensor_tensor(out=g32[:], in0=g32[:], in1=sm["s1", kt][:],
                                    op=OP.mult)
            dekker(g_t, 2 * ver, g32, f"dkg{kt}{ver}")
        gA[kt] = g_t

    # ------------- phase 3: per-sample masks + weighted sums ---------------
    for t_ps in acc_ps:
        nc.vector.memset(t_ps[:], 0.0)
    stage2 = cp.tile([128, 20 * 128], F32, tag="stage2", name="t")
    pdi = 0

    tqp_tiles = {}
    mask_store = {}

    def dma_tqpbc(s):
        t = qp.tile([128, Q], F32, tag="tqpbc", name="t")
        nc.sync.dma_start(out=t[:], in_=bcast(queriesp_d[s, :]))
        tqp_tiles[s] = t

    def emit_masks(s):
        nonlocal pdi
        tqpbc = tqp_tiles.pop(s)
        masks = {}
        for kt in range(2):
            # mA/mC: one fused ACT sigmoid-mask op each
            for grp, a in (("A", 1), ("C", 3)):
                m = mp.tile([128, Q], BF16, tag=f"m{grp}{kt}", name="t")
                nc.scalar.activation(m[:], tqpbc[:], ACTF.Sigmoid,
                                     bias=negS[a, kt][:, s:s + 1], scale=SCALE)
                masks[grp, kt] = m
            # mD/mE: PE split-diffs (PSUM) + DVE compares (mE needs the
            # exact-0 self-compare at head queries)
            for grp, stat, op in (("B", statB[kt], OP.is_gt),
                                  ("D", statD[kt], OP.is_ge),
                                  ("E", statE[kt], OP.is_gt)):
                pd = pdw_ps[pdi % 3]
                pdi += 1
                nc.tensor.matmul(pd[:, 0:Q], stk(stat, s, 128),
                                 stk(rhsQ, s, Q), start=True, stop=True)
                m = mp.tile([128, Q], BF16, tag=f"m{grp}{kt}", name="t")
                nc.vector.tensor_scalar(out=m[:], in0=pd[:, 0:Q], scalar1=0.0,
                                        scalar2=None, op0=op)
                masks[grp, kt] = m
        mask_store[s] = masks

    def emit_consumers(s):
        # group g accumulates at acc rows 32g..32g+4 (one rotating bank)
        masks = mask_store.pop(s)
        acc = acc_ps[s % 4]
        for kt in range(2):
            st = (kt == 0)
            sp_ = (kt == 1)
            nc.tensor.matmul(acc[0:4, 0:Q], gA[kt][:, s::S][:, 0:4],
                             masks["A", kt][:], start=st, stop=sp_)
            nc.tensor.matmul(acc[32:36, 0:Q], vB[kt][:, s::S][:, 0:4],
                             masks["B", kt][:], start=st, stop=sp_)
            nc.tensor.matmul(acc[64:68, 0:Q], vC[kt][:, s::S][:, 0:4],
                             masks["C", kt][:], start=st, stop=sp_)
            # D'+E share rows 96-100 (E via the zero-padded dsh lhsT)
            nc.tensor.matmul(acc[96:101, 0:Q], dsh[kt][:, s::S][:, 0:5],
                             masks["E", kt][:], start=st, stop=sp_,
                             tile_position=(0, 96))
            nc.tensor.matmul(acc[96:100, 0:Q], vC[kt][:, s::S][:, 0:4],
                             masks["D", kt][:], start=False, stop=False,
                             skip_group_check=True, tile_position=(0, 96))

    def emit_drain(s):
        # drain acc row-group 32g -> stga col-block g (all starts aligned;
        # PSUM reads only on ACT/DVE), then scatter to stage2 rows (32qt+s)
        acc = acc_ps[s % 4]
        stga = gp.tile([5, 4 * Q], F32, tag="stga", name="t")
        nc.scalar.copy(stga[0:5, 0:Q], acc[0:5, 0:Q])
        for g in range(1, 4):
            nc.vector.tensor_copy(out=stga[0:5, g * Q:(g + 1) * Q],
                                  in_=acc[32 * g:32 * g + 5, 0:Q])
        srcv = stga[:].rearrange("p (g x) -> p g x", g=4)
        for qt in range(4):
            row = 32 * qt + s
            dst = stage2[row:row + 1, :].rearrange(
                "one (j g c) -> one j g c", j=5, g=4)
            eng = nc.sync if qt < 1 else nc.gpsimd
            eng.dma_start(out=dst[:, :, :, :],
                          in_=srcv[:, :, qt * 128:(qt + 1) * 128])

    # software pipeline: tqpbc prefetch (+2), masks (lag 0), consumers
    # (lag 1), drain+scatter (lag 2) — keeps ACT/PE/DVE streams decoupled
    dma_tqpbc(0)
    dma_tqpbc(1)
    dma_tqpbc(2)
    for i in range(S + 2):
        if i < S:
            if i + 3 < S:
                dma_tqpbc(i + 3)
            emit_masks(i)
        if 0 <= i - 1 < S:
            emit_consumers(i - 1)
        if 0 <= i - 2 < S:
            emit_drain(i - 2)

    # ------------- phase 4: batched post-processing ------------------------
    def R(r):
        return stage2[:, r * 128:(r + 1) * 128]

    # query matrix [128 (qt,s), 128]: tq_m[32qt+s, c] = queries[s, 128qt+c]
    tq_m = cp.tile([128, 128], F32, tag="tqm", name="t")
    for qt in range(4):
        nc.sync.dma_start(out=tq_m[32 * qt:32 * (qt + 1), :],
                          in_=queries_d[:, 128 * qt:128 * (qt + 1)])

    def tmp(tag):
        return cp.tile([128, 128], F32, tag=tag, name="t")

    # role r = j*4 + g (j = quad row [C1h C1l C2h C2l], g = group A B C' D');
    # j=4: roles 16-18 junk-zero, 19 = E
    for r in (0, 1, 2, 3, 8, 9, 10, 11):
        nc.vector.tensor_tensor(out=R(r), in0=R(r), in1=R(r + 4), op=OP.add)
    A1, B1, Cs1, Dr1, A2, B2, Cs2, Dr2 = (R(r) for r in (0, 1, 2, 3, 8, 9, 10, 11))

    blk = cp.tile([128, 128], U8, tag="blk", name="t")
    nc.vector.tensor_scalar(out=blk[:], in0=tq_m[:], scalar1=C1, scalar2=None,
                            op0=OP.is_ge)
    biasC1 = cp.tile([128, 1], F32, tag="biasC1", name="t")
    nc.vector.memset(biasC1[:], C1)
    biasC2 = cp.tile([128, 1], F32, tag="biasC2", name="t")
    nc.vector.memset(biasC2[:], C2)
    e1 = tmp("e1")
    nc.scalar.activation(e1[:], tq_m[:], ACTF.Exp, bias=biasC1[:], scale=-1.0)
    e2 = tmp("e2")
    nc.scalar.activation(e2[:], tq_m[:], ACTF.Exp, bias=biasC2[:], scale=-1.0)

    def sel(tag, on_true, on_false):
        o = tmp(tag)
        nc.vector.select(o, blk[:], on_true, on_false)
        return o

    esel = sel("esel", e2[:], e1[:])
    Asel = sel("Asel", A2, A1)
    Bsel = sel("Bsel", B2, B1)
    Csel = sel("Csel", Cs2, Cs1)
    Dsel = sel("Dsel", Dr2, Dr1)

    feat0 = tmp("feat0")
    nc.vector.tensor_tensor(out=feat0[:], in0=esel[:], in1=Asel[:], op=OP.mult)
    nc.vector.tensor_tensor(out=feat0[:], in0=feat0[:], in1=esel[:], op=OP.mult)
    feat1 = tmp("feat1")
    nc.vector.tensor_tensor(out=feat1[:], in0=esel[:], in1=Bsel[:], op=OP.mult)
    feat2 = tmp("feat2")
    nc.vector.tensor_tensor(out=feat2[:], in0=Dsel[:], in1=Csel[:], op=OP.subtract)
    nc.vector.tensor_tensor(out=feat2[:], in0=feat2[:], in1=esel[:], op=OP.mult)

    eff0 = tmp("eff0")
    nc.vector.tensor_scalar(out=eff0[:], in0=R(19), scalar1=-2.0,
                            scalar2=escol[:], op0=OP.mult, op1=OP.add)

    combo = tmp("combo")
    nc.vector.tensor_scalar(out=combo[:], in0=feat0[:], scalar1=wbbc[:, 0:1],
                            scalar2=None, op0=OP.mult)
    nc.vector.scalar_tensor_tensor(out=combo[:], in0=feat1[:], scalar=wbbc[:, 1:2],
                                   in1=combo[:], op0=OP.mult, op1=OP.add)
    nc.vector.scalar_tensor_tensor(out=combo[:], in0=feat2[:], scalar=negw2[:],
                                   in1=combo[:], op0=OP.mult, op1=OP.add)
    logits = tmp("logits")
    nc.vector.tensor_tensor(out=logits[:], in0=combo[:], in1=eff0[:], op=OP.mult)
    nc.vector.tensor_scalar(out=logits[:], in0=logits[:], scalar1=wbbc[:, 3:4],
                            scalar2=None, op0=OP.add)
    # zero the sentinel query (qt==1 rows, col 127)
    nc.vector.tensor_tensor(out=logits[:, 127:128], in0=logits[:, 127:128],
                            in1=padcol, op=OP.mult)

    hsum = cp.tile([128, 1], F32, tag="hsum", name="t")
    nc.vector.tensor_reduce(out=hsum[:], in_=logits[:], axis=AX.X, op=OP.add)
    expt = tmp("expt")
    intcol = cp.tile([128, 1], F32, tag="intcol", name="t")
    nc.scalar.activation(expt[:], logits[:], ACTF.Exp, accum_out=intcol[:])
    nc.vector.tensor_scalar(out=intcol[:], in0=intcol[:], scalar1=-RES,
                            scalar2=None, op0=OP.mult)
    qtmaski = cp.tile([128, 1], U8, tag="qtmaski", name="t")
    nc.vector.tensor_scalar(out=qtmaski[:], in0=qtmask, scalar1=0.5,
                            scalar2=None, op0=OP.is_ge)
    rowpart = cp.tile([128, 1], F32, tag="rowpart", name="t")
    nc.vector.select(rowpart[:], qtmaski[:], hsum[:], intcol[:])
    nc.sync.dma_start(out=out_d[:], in_=rowpart[:])

    for pool in (pp, gp, mp, qp, sp, cp):
        pool.release()


_NC_CACHE = []


def _get_nc():
    if not _NC_CACHE:
        _NC_CACHE.append(build_nc())
    return _NC_CACHE[0]


def make_inputs_for_core(times, states, base, weights, core):
    grid = np.arange(0.0, T_MAX, RES, dtype=np.float32)
    consts = np.ones((128, 2), np.float32)
    consts[64:128, 0] = 0.0   # qtmask: 0 for grid rows (qt 2,3 blocks)
    consts[32:64, 1] = 0.0    # sentinel-column mask: 0 for qt1 block
    sl = slice(core * S, (core + 1) * S)
    t = np.ascontiguousarray(times[sl]).astype(np.float32)
    st = np.ascontiguousarray(states[sl]).astype(np.int32)
    queries = np.concatenate(
        [t[:, 4, 1:256], np.full((S, 1), BIG, np.float32),
         np.tile(grid, (S, 1))], axis=1).astype(np.float32)
    queriesp = (queries - np.float32(TOL)).astype(np.float32)
    times1p = (t[:, 1, :] - np.float32(TOL)).astype(np.float32)
    tT = np.ascontiguousarray(t.transpose(1, 2, 0).reshape(5, 2, EH, S))
    sT = np.ascontiguousarray(st.transpose(1, 2, 0).reshape(5, 2, EH, S))
    return {
        "timesT": tT,
        "statesT": sT,
        "base": np.asarray(base, np.float32),
        "weights": np.asarray(weights, np.float32),
        "queries": queries,
        "queriesp": queriesp,
        "times1p": times1p,
        "onesbf": np.ones(S * Q, mybir.dt.np(BF16)),
        "consts": consts,
    }


def kernel(times, states, base, weights):
    from concourse.bass_utils import run_bass_kernel_spmd

    times = np.asarray(times, np.float32)
    states = np.asarray(states, np.int32)
    nc = _get_nc()
    in_maps = [make_inputs_for_core(times, states, base, weights, c)
               for c in range(NCORES)]
    res = run_bass_kernel_spmd(nc, in_maps, list(range(NCORES)))
    parts = np.stack([np.asarray(res.results[c]["out"]) for c in range(NCORES)])
    total = np.sum(parts.astype(np.float32), dtype=np.float32)
    return np.array([total], dtype=np.float32)


def run_traced(times, states, base, weights):
    """Profiled run; returns BassKernelResults (exec_time_ns etc.)."""
    from concourse.bass_utils import run_bass_kernel_spmd

    times = np.asarray(times, np.float32)
    states = np.asarray(states, np.int32)
    nc = _get_nc()
    in_maps = [make_inputs_for_core(times, states, base, weights, c)
               for c in range(NCORES)]
    res = run_bass_kernel_spmd(nc, in_maps, list(range(NCORES)), trace=True)
    return res


# revision 65
# speedup vs baseline: 1.2664x; 1.1104x over previous
"""Trainium2 Bass kernel for nn_Logic_Learning_Model (declarative logic-rule
point-process log-likelihood).

Algorithm (factorized; validated vs reference at ~4e-7 rel err in numpy):
For each sample, all features are masked weighted sums over per-predicate
event arrays evaluated at 512 query times (255 head-event times + 1 big
sentinel + 256 grid points):

  feat0(q) = e^{2(Ck-tq)} * sum_j [t1_j < tq-TOL] * g_j(Ck)
             g_j(Ck) = [s1_j==1] * e^{2(t1_j-Ck)} * What_j
             What_j  = e^{C2-t1_j} * sum_i [t0_i < t1_j-TOL][s0_i==1] e^{t0_i-C2}
  feat1(q) = e^{Ck-tq} * sum_j [t2_j < tq-TOL] * [s2_j==1] e^{t2_j-Ck}
  feat2(q) = e^{Ck-tq} * ( D'(q) - C'(q) ),  D' = sum [t3<=tq] v3,
             C' = sum [t3<tq-TOL] v3,  v3_j = [s3_j==0] e^{t3_j-Ck}
  sh[idx(q)] = sum_j [tq > th_j] * (sh_j - sh_{j-1,wrap}) + sh_255

Ck is a per-query-block shift (C1=38.4 for tq<38.4, C2=76.8 otherwise) to
keep every exponential inside fp32 range.  Masks are exact 0/1 bf16 tiles:
 - mA/mB/mC: ACT-engine Identity activation computes diff = fl(tq-TOL) - t
   via the fast per-partition bias path; DVE/Pool immediate-scalar compares
   produce the 0/1 masks (exact fp32 reference rounding).
 - mD/mE/mwt: PE outer-product diffs from exact bf16 triple-splits
   (hi+mid+lo == fp32 value), compared on Pool/DVE.
Weighted sums run on the PE as bf16 matmuls with Dekker-split (hi+lo)
weight vectors accumulating in fp32 PSUM; all five groups share one PSUM
bank per sample (rows 0-3 A, 32-35 B, 64-67 C', 96-99 D', 100 E).

Sharding: pure data parallel, 32 samples per core on 8 cores; each core
returns 128 per-(sample,query-tile) partial sums; host adds them up.
"""

import numpy as np

import concourse.bass as bass
import concourse.mybir as mybir
from concourse.tile import TileContext

F32 = mybir.dt.float32
BF16 = mybir.dt.bfloat16
I32 = mybir.dt.int32
U8 = mybir.dt.uint8

NCORES = 8
S = 32          # samples per core
E = 256         # events per predicate
EH = 128        # half (one partition tile)
Q = 512         # query count: 255 head + 1 big sentinel + 256 grid
T_MAX = 76.8
RES = 0.3
TOL = 0.1
C1 = 38.4
C2 = 76.8
BIG = 1000.0

AX = mybir.AxisListType
OP = mybir.AluOpType
ACTF = mybir.ActivationFunctionType


def bcast(ap, n=128):
    """0-stride partition broadcast view of a flat DRAM AP."""
    return bass.AP(ap.tensor, ap.offset, [[0, n]] + list(ap.ap))


def build_nc():
    from concourse.bacc import Bacc
    nc = Bacc(None, target_bir_lowering=False)
    timesT_d = nc.dram_tensor("timesT", [5, 2, EH, S], F32, kind="ExternalInput")
    statesT_d = nc.dram_tensor("statesT", [5, 2, EH, S], I32, kind="ExternalInput")
    base_d = nc.dram_tensor("base", [1], F32, kind="ExternalInput")
    weights_d = nc.dram_tensor("weights", [3], F32, kind="ExternalInput")
    # queries[s] = [th[1:256], BIG, grid];  queriesp = fl(queries - TOL)
    queries_d = nc.dram_tensor("queries", [S, Q], F32, kind="ExternalInput")
    queriesp_d = nc.dram_tensor("queriesp", [S, Q], F32, kind="ExternalInput")
    times1p_d = nc.dram_tensor("times1p", [S, E], F32, kind="ExternalInput")
    onesbf_d = nc.dram_tensor("onesbf", [S * Q], BF16, kind="ExternalInput")
    # consts[:, 0] = qtmask (1 for head rows), consts[:, 1] = pad column mask
    consts_d = nc.dram_tensor("consts", [128, 2], F32, kind="ExternalInput")
    out_d = nc.dram_tensor("out", [128], F32, kind="ExternalOutput")

    with TileContext(nc) as tc:
        _build(tc, nc, timesT_d, statesT_d, base_d, weights_d, queries_d,
               queriesp_d, times1p_d, onesbf_d, consts_d, out_d)
    nc.finalize()
    return nc


def _build(tc, nc, timesT_d, statesT_d, base_d, weights_d, queries_d,
           queriesp_d, times1p_d, onesbf_d, consts_d, out_d):
    cp = tc.alloc_tile_pool(name="const", bufs=1)
    sp = tc.alloc_tile_pool(name="scr", bufs=3)
    s1 = tc.alloc_tile_pool(name="scr1", bufs=1)
    pp = tc.alloc_tile_pool(name="psum", bufs=1, space="PSUM")

    # ---------------- phase 0: loads + batched prep ----------------
    T = {}
    ST = {}
    for a in range(5):
        for kt in range(2):
            t_t = cp.tile([EH, S], F32, tag=f"T{a}{kt}", name="t")
            nc.sync.dma_start(out=t_t[:], in_=timesT_d[a, kt])
            T[a, kt] = t_t
            s_t = cp.tile([EH, S], I32, tag=f"S{a}{kt}", name="t")
            nc.sync.dma_start(out=s_t[:], in_=statesT_d[a, kt])
            ST[a, kt] = s_t

    # base/weights broadcast columns
    wbbc = cp.tile([128, 4], F32, tag="wbbc", name="t")
    nc.vector.memset(wbbc[:], 0.0)
    nc.sync.dma_start(out=wbbc[:, 0:3], in_=bcast(weights_d[:]))
    nc.sync.dma_start(out=wbbc[:, 3:4], in_=bcast(base_d[:]))
    negw2 = cp.tile([128, 1], F32, tag="negw2", name="t")
    nc.vector.tensor_scalar(out=negw2[:], in0=wbbc[:, 2:3], scalar1=-1.0,
                            scalar2=None, op0=OP.mult)

    consts = cp.tile([128, 2], F32, tag="consts", name="t")
    nc.sync.dma_start(out=consts[:], in_=consts_d[:])
    qtmask = consts[:, 0:1]
    padcol = consts[:, 1:2]

    # sigmoid-mask bias columns: -SCALE*t.  ACT computes
    # sigmoid(SCALE*tq + bias) which saturates to exactly 0.0/1.0 in fp32,
    # fusing diff+compare into one ACT op (threshold noise ~eps*t, negligible).
    SCALE = 1.0e20
    negS = {}
    for a in (0, 1, 2, 3):
        for kt in range(2):
            m = cp.tile([EH, S], F32, tag=f"negS{a}{kt}", name="t")
            nc.vector.tensor_scalar(out=m[:], in0=T[a, kt][:], scalar1=-SCALE,
                                    scalar2=None, op0=OP.mult)
            negS[a, kt] = m

    # batched exponentials / state masks
    ew = {}
    sm = {}
    for kt in range(2):
        def _exp(tag, src, scale, off):
            arg = sp.tile([EH, S], F32, tag=f"arg{tag}{kt}", name="t")
            nc.vector.tensor_scalar(out=arg[:], in0=src[:], scalar1=scale,
                                    scalar2=off, op0=OP.mult, op1=OP.add)
            e_t = cp.tile([EH, S], F32, tag=f"e{tag}{kt}", name="t")
            nc.scalar.activation(e_t[:], arg[:], ACTF.Exp)
            return e_t

        ew["w0", kt] = _exp("w0", T[0, kt], 1.0, -C2)
        ew["c2t1", kt] = _exp("c2t1", T[1, kt], -1.0, C2)
        ew["g1", kt] = _exp("g1", T[1, kt], 2.0, -2.0 * C1)
        ew["g2", kt] = _exp("g2", T[1, kt], 2.0, -2.0 * C2)
        ew["v21", kt] = _exp("v21", T[2, kt], 1.0, -C1)
        ew["v22", kt] = _exp("v22", T[2, kt], 1.0, -C2)
        ew["v31", kt] = _exp("v31", T[3, kt], 1.0, -C1)
        ew["v32", kt] = _exp("v32", T[3, kt], 1.0, -C2)

        for a, val, tag in ((0, 1, "s0"), (1, 1, "s1"), (2, 1, "s2"), (3, 0, "s3")):
            m = cp.tile([EH, S], F32, tag=f"{tag}{kt}", name="t")
            nc.vector.tensor_scalar(out=m[:], in0=ST[a, kt][:], scalar1=val,
                                    scalar2=None, op0=OP.is_equal)
            sm[tag, kt] = m

        # [t3 <= C1]: zero v3C1 entries no C1-block query can select (keeps
        # PSUM partial sums small)
        m31 = cp.tile([EH, S], F32, tag=f"m31{kt}", name="t")
        nc.vector.tensor_scalar(out=m31[:], in0=T[3, kt][:], scalar1=C1,
                                scalar2=None, op0=OP.is_le)
        sm["m31", kt] = m31

    def dekker(dst, blk0, src32, tmp_tag, eng=None):
        """write bf16 (hi, lo) blocks of src32 [128, S] into dst block cols"""
        eng = eng or nc.vector
        hi = dst[:, blk0 * S:(blk0 + 1) * S]
        lo = dst[:, (blk0 + 1) * S:(blk0 + 2) * S]
        eng.tensor_copy(out=hi, in_=src32[:])
        tmp = sp.tile([EH, S], F32, tag=tmp_tag, name="t")
        eng.tensor_copy(out=tmp[:], in_=hi)
        eng.tensor_tensor(out=lo, in0=src32[:], in1=tmp[:], op=OP.subtract)

    # w0 pairs (feat0 inner sum weights)
    w0pair = {}
    for kt in range(2):
        w0 = sp.tile([EH, S], F32, tag=f"w0m{kt}", name="t")
        nc.vector.tensor_tensor(out=w0[:], in0=ew["w0", kt][:], in1=sm["s0", kt][:],
                                op=OP.mult)
        pair = cp.tile([EH, 2 * S], BF16, tag=f"w0pair{kt}", name="t")
        dekker(pair, 0, w0, f"w0tmp{kt}")
        w0pair[kt] = pair

    # v2 / v3 quads [128, 4*S]: cols 4s..4s+3 = [vC1h vC1l vC2h vC2l]
    vB = {}
    vC = {}
    for kt in range(2):
        q_b = cp.tile([EH, 4 * S], BF16, tag=f"vB{kt}", name="t")
        q_c = cp.tile([EH, 4 * S], BF16, tag=f"vC{kt}", name="t")
        for ver, (e2tag, e3tag) in enumerate((("v21", "v31"), ("v22", "v32"))):
            v2 = sp.tile([EH, S], F32, tag=f"v2m{kt}{ver}", name="t")
            nc.gpsimd.tensor_tensor(out=v2[:], in0=ew[e2tag, kt][:],
                                    in1=sm["s2", kt][:], op=OP.mult)
            dekker(q_b, 2 * ver, v2, f"dkb{kt}{ver}", eng=nc.gpsimd)
            v3 = sp.tile([EH, S], F32, tag=f"v3m{kt}{ver}", name="t")
            nc.gpsimd.tensor_tensor(out=v3[:], in0=ew[e3tag, kt][:],
                                    in1=sm["s3", kt][:], op=OP.mult)
            if ver == 0:
                nc.gpsimd.tensor_tensor(out=v3[:], in0=v3[:],
                                        in1=sm["m31", kt][:], op=OP.mult)
            dekker(q_c, 2 * ver, v3, f"dkc{kt}{ver}", eng=nc.gpsimd)
        vB[kt] = q_b
        vC[kt] = q_c

    # dsh (bf16): sh_j - sh_{j-1 (wrap)}; zero-padded [z z z z dsh]
    shm1 = {0: sp.tile([EH, S], I32, tag="shm10", name="t"),
            1: sp.tile([EH, S], I32, tag="shm11", name="t")}
    nc.vector.memset(shm1[0][:], 0)
    nc.vector.memset(shm1[1][:], 0)
    nc.sync.dma_start(out=shm1[0][1:128, :], in_=ST[4, 0][0:127, :])
    nc.sync.dma_start(out=shm1[0][0:1, :], in_=ST[4, 1][127:128, :])
    nc.sync.dma_start(out=shm1[1][1:128, :], in_=ST[4, 1][0:127, :])
    nc.sync.dma_start(out=shm1[1][0:1, :], in_=ST[4, 0][127:128, :])
    dsh = {}
    for kt in range(2):
        d = cp.tile([EH, 5 * S], BF16, tag=f"dsh{kt}", name="t")
        nc.vector.memset(d[:], 0.0)
        nc.vector.tensor_tensor(out=d[:, 4 * S:5 * S], in0=ST[4, kt][:],
                                in1=shm1[kt][:], op=OP.subtract)
        dsh[kt] = d

    # escol = 1 - 2*sh[255], per (sample,qt) partition column
    sh255row = sp.tile([1, S], I32, tag="sh255row", name="t")
    nc.sync.dma_start(out=sh255row[:], in_=ST[4, 1][127:128, :])
    esrow = cp.tile([1, S], F32, tag="esrow", name="t")
    nc.vector.tensor_scalar(out=esrow[:], in0=sh255row[:], scalar1=-2.0,
                            scalar2=1.0, op0=OP.mult, op1=OP.add)
    escol = cp.tile([128, 1], F32, tag="escol", name="t")
    nc.vector.memset(escol[:], 0.0)
    for qt in range(4):
        nc.sync.dma_start(out=escol[32 * qt:32 * (qt + 1), :], in_=esrow[0:1, :])

    # identity for PE transposes
    iot = sp.tile([128, 128], I32, tag="iot", name="t")
    nc.gpsimd.iota(iot[:], pattern=[[1, 128]], base=0, channel_multiplier=-1)
    ident = cp.tile([128, 128], F32, tag="ident", name="t")
    nc.vector.tensor_scalar(out=ident[:], in0=iot[:], scalar1=0,
                            scalar2=None, op0=OP.is_equal)

    # PSUM tiles (8 banks: acc0-3 | pd0-2 | psw); the transpose scratch
    # reuses pd tiles before any diff runs.  Each acc bank holds all four
    # accumulation groups at row offsets 0/32/64/96 (A, B, C', D'+E).
    psw = pp.tile([128, 4], F32, tag="psw", name="t")
    pdw_ps = [pp.tile([128, Q], F32, tag=f"pd{i}", name="t") for i in range(3)]
    acc_ps = [pp.tile([128, Q], F32, tag=f"acc{i}", name="t") for i in range(4)]

    # ---- triple-split helpers ----
    def split3(dst_list, src_ap, part, cols, eng):
        """exact fp32 = hi+mid+lo bf16 split; dst_list = 3 bf16 tiles"""
        hi, mid, lo = dst_list
        r1 = s1.tile([part, cols], F32, tag=f"s3r1_{part}_{cols}", name="t")
        r1f = s1.tile([part, cols], F32, tag=f"s3rf_{part}_{cols}", name="t")
        eng.tensor_copy(out=hi[:], in_=src_ap)
        eng.tensor_copy(out=r1f[:], in_=hi[:])
        eng.tensor_tensor(out=r1[:], in0=src_ap, in1=r1f[:], op=OP.subtract)
        eng.tensor_copy(out=mid[:], in_=r1[:])
        eng.tensor_copy(out=r1f[:], in_=mid[:])
        eng.tensor_tensor(out=r1[:], in0=r1[:], in1=r1f[:], op=OP.subtract)
        eng.tensor_copy(out=lo[:], in_=r1[:])

    # query rows + their splits: qrow [32, Q] (tq)
    qrow = s1.tile([S, Q], F32, tag="qrow", name="t")
    nc.sync.dma_start(out=qrow[:], in_=queries_d[:])

    qspl = [s1.tile([S, Q], BF16, tag=f"qspl{k}", name="t") for k in range(3)]
    split3(qspl, qrow[:], S, Q, nc.vector)

    # negated transposed event splits for the PE-diff stationaries:
    # t2+TOL (mB), t3 (mD), t4 (mE)
    ttspl = {}
    for i, (a, kt) in enumerate([(3, 0), (3, 1), (4, 0), (4, 1)]):
        ps = pdw_ps[i % 3]
        nc.tensor.transpose(ps[0:S, 0:128], T[a, kt][:], ident[:])
        tt = s1.tile([S, 128], F32, tag=f"tt{i%2}", name="t")
        nc.scalar.copy(tt[:], ps[0:S, 0:128])
        ntt = s1.tile([S, 128], F32, tag=f"ntt{i%2}", name="t")
        # mB compares against tq (not tq-TOL): fold TOL into the t2 side
        off = -TOL if a == 2 else 0.0
        nc.vector.tensor_scalar(out=ntt[:], in0=tt[:], scalar1=-1.0,
                                scalar2=off, op0=OP.mult, op1=OP.add)
        spl = [s1.tile([S, 128], BF16, tag=f"nts{a}{kt}{k}", name="t")
               for k in range(3)]
        split3(spl, ntt[:], S, 128, nc.vector)
        ttspl[a, kt] = spl

    # 6-partition stacks, free dim = sample-major:
    #   stationary [6, S*ncols]: rows 0-2 = -splits, rows 3-5 = 1
    #   rhs        [6, S*ncols]: rows 0-2 = 1, rows 3-5 = +query splits
    # per-sample operand = [0:6, s*ncols:(s+1)*ncols]  (base partition 0);
    # ones rows come from DRAM (engine memsets can't start at partition 3).
    def stack6(tagbase, ncols, split_src, neg_first):
        t = cp.tile([6, S * ncols], BF16, tag=tagbase, name="t")
        r0 = 0 if neg_first else 3
        o0 = 3 if neg_first else 0
        nc.gpsimd.dma_start(out=t[o0:o0 + 3, :],
                            in_=bcast(onesbf_d[0:S * ncols], 3))
        for k in range(3):
            nc.gpsimd.dma_start(out=t[r0 + k:r0 + k + 1, :],
                                in_=split_src[k][0:S, 0:ncols])
        return t

    statD = {}
    statE = {}
    for kt in range(2):
        statD[kt] = stack6(f"stD{kt}", 128, ttspl[3, kt], True)
        statE[kt] = stack6(f"stE{kt}", 128, ttspl[4, kt], True)
    rhsQ = stack6("rhQ", Q, qspl, False)
    s1.release()

    # per-sample pools allocated after s1's release so they reuse its space
    qp = tc.alloc_tile_pool(name="qbc", bufs=4)
    mp = tc.alloc_tile_pool(name="mask", bufs=4)
    gp = tc.alloc_tile_pool(name="stga", bufs=2)

    def stk(tile, s, ncols):
        return tile[0:6, s * ncols:(s + 1) * ncols]

    # ------------- phase 1: per-sample What (feat0 inner sums) -------------
    wst = cp.tile([128, 4 * S], F32, tag="wst", name="t")
    t1p_tiles = {}
    mwt_store = {}

    def dma_t1pbc(s):
        t = qp.tile([128, E], F32, tag="t1pbc", name="t")
        nc.sync.dma_start(out=t[:], in_=bcast(times1p_d[s, :]))
        t1p_tiles[s] = t

    dma_t1pbc(0)
    dma_t1pbc(1)
    for i in range(S + 1):
        if i < S:
            if i + 2 < S:
                dma_t1pbc(i + 2)
            mwts = []
            for ikt in range(2):
                mwt = mp.tile([128, E], BF16, tag=f"mwt{ikt}", name="t")
                nc.scalar.activation(mwt[:], t1p_tiles[i][:], ACTF.Sigmoid,
                                     bias=negS[0, ikt][:, i:i + 1], scale=SCALE)
                mwts.append(mwt)
            mwt_store[i] = mwts
        s = i - 1
        if 0 <= s:
            mwts = mwt_store.pop(s)
            for jkt in range(2):
                for ikt in range(2):
                    nc.tensor.matmul(psw[:, 2 * jkt:2 * jkt + 2],
                                     mwts[ikt][:, jkt * EH:(jkt + 1) * EH],
                                     w0pair[ikt][:, s::S][:, 0:2],
                                     start=(ikt == 0), stop=(ikt == 1))
            nc.vector.tensor_copy(out=wst[:, s::S][:, 0:4], in_=psw[:])

    # ------------- phase 2: batched g-vector assembly (feat0 weights) ------
    gA = {}
    for kt in range(2):
        wh = sp.tile([EH, S], F32, tag=f"wh{kt}", name="t")
        nc.vector.tensor_tensor(out=wh[:], in0=wst[:, 2 * kt * S:(2 * kt + 1) * S],
                                in1=wst[:, (2 * kt + 1) * S:(2 * kt + 2) * S],
                                op=OP.add)
        nc.vector.tensor_tensor(out=wh[:], in0=wh[:], in1=ew["c2t1", kt][:],
                                op=OP.mult)
        g_t = cp.tile([EH, 4 * S], BF16, tag=f"gA{kt}", name="t")
        for ver, etag in enumerate(("g1", "g2")):
            g32 = sp.tile([EH, S], F32, tag=f"g32{kt}{ver}", name="t")
            nc.vector.tensor_tensor(out=g32[:], in0=ew[etag, kt][:], in1=wh[:],
                                    op=OP.mult)
            nc.vector.tensor_tensor(out=g32[:], in0=g32[:], in1=sm["s1", kt][:],
                                    op=OP.mult)
            dekker(g_t, 2 * ver, g32, f"dkg{kt}{ver}")
        gA[kt] = g_t

    # ------------- phase 3: per-sample masks + weighted sums ---------------
    for t_ps in acc_ps:
        nc.vector.memset(t_ps[:], 0.0)
    stage2 = cp.tile([128, 20 * 128], F32, tag="stage2", name="t")
    pdi = 0

    tqp_tiles = {}
    mask_store = {}

    def dma_tqpbc(s):
        t = qp.tile([128, Q], F32, tag="tqpbc", name="t")
        nc.sync.dma_start(out=t[:], in_=bcast(queriesp_d[s, :]))
        tqp_tiles[s] = t

    def emit_masks(s):
        nonlocal pdi
        tqpbc = tqp_tiles.pop(s)
        masks = {}
        for kt in range(2):
            # mA/mB/mC: one fused ACT sigmoid-mask op each
            for grp, a in (("A", 1), ("B", 2), ("C", 3)):
                m = mp.tile([128, Q], BF16, tag=f"m{grp}{kt}", name="t")
                nc.scalar.activation(m[:], tqpbc[:], ACTF.Sigmoid,
                                     bias=negS[a, kt][:, s:s + 1], scale=SCALE)
                masks[grp, kt] = m
            # mD/mE: PE split-diffs (PSUM) + DVE compares (mE needs the
            # exact-0 self-compare at head queries)
            for grp, stat, op in (("D", statD[kt], OP.is_ge),
                                  ("E", statE[kt], OP.is_gt)):
                pd = pdw_ps[pdi % 3]
                pdi += 1
                nc.tensor.matmul(pd[:, 0:Q], stk(stat, s, 128),
                                 stk(rhsQ, s, Q), start=True, stop=True)
                m = mp.tile([128, Q], BF16, tag=f"m{grp}{kt}", name="t")
                nc.vector.tensor_scalar(out=m[:], in0=pd[:, 0:Q], scalar1=0.0,
                                        scalar2=None, op0=op)
                masks[grp, kt] = m
        mask_store[s] = masks

    def emit_consumers(s):
        # group g accumulates at acc rows 32g..32g+4 (one rotating bank)
        masks = mask_store.pop(s)
        acc = acc_ps[s % 4]
        for kt in range(2):
            st = (kt == 0)
            sp_ = (kt == 1)
            nc.tensor.matmul(acc[0:4, 0:Q], gA[kt][:, s::S][:, 0:4],
                             masks["A", kt][:], start=st, stop=sp_)
            nc.tensor.matmul(acc[32:36, 0:Q], vB[kt][:, s::S][:, 0:4],
                             masks["B", kt][:], start=st, stop=sp_)
            nc.tensor.matmul(acc[64:68, 0:Q], vC[kt][:, s::S][:, 0:4],
                             masks["C", kt][:], start=st, stop=sp_)
            # D'+E share rows 96-100 (E via the zero-padded dsh lhsT)
            nc.tensor.matmul(acc[96:101, 0:Q], dsh[kt][:, s::S][:, 0:5],
                             masks["E", kt][:], start=st, stop=sp_,
                             tile_position=(0, 96))
            nc.tensor.matmul(acc[96:100, 0:Q], vC[kt][:, s::S][:, 0:4],
                             masks["D", kt][:], start=False, stop=False,
                             skip_group_check=True, tile_position=(0, 96))

    def emit_drain(s):
        # drain acc row-group 32g -> stga col-block g (all starts aligned;
        # PSUM reads only on ACT/DVE), then scatter to stage2 rows (32qt+s)
        acc = acc_ps[s % 4]
        stga = gp.tile([5, 4 * Q], F32, tag="stga", name="t")
        for g in range(4):
            nc.vector.tensor_copy(out=stga[0:5, g * Q:(g + 1) * Q],
                                  in_=acc[32 * g:32 * g + 5, 0:Q])
        srcv = stga[:].rearrange("p (g x) -> p g x", g=4)
        for qt in range(4):
            row = 32 * qt + s
            dst = stage2[row:row + 1, :].rearrange(
                "one (j g c) -> one j g c", j=5, g=4)
            eng = nc.sync if qt < 2 else nc.gpsimd
            eng.dma_start(out=dst[:, :, :, :],
                          in_=srcv[:, :, qt * 128:(qt + 1) * 128])

    # software pipeline: tqpbc prefetch (+2), masks (lag 0), consumers
    # (lag 1), drain+scatter (lag 2) — keeps ACT/PE/DVE streams decoupled
    dma_tqpbc(0)
    dma_tqpbc(1)
    dma_tqpbc(2)
    for i in range(S + 2):
        if i < S:
            if i + 3 < S:
                dma_tqpbc(i + 3)
            emit_masks(i)
        if 0 <= i - 1 < S:
            emit_consumers(i - 1)
        if 0 <= i - 2 < S:
            emit_drain(i - 2)

    # ------------- phase 4: batched post-processing ------------------------
    def R(r):
        return stage2[:, r * 128:(r + 1) * 128]

    # query matrix [128 (qt,s), 128]: tq_m[32qt+s, c] = queries[s, 128qt+c]
    tq_m = cp.tile([128, 128], F32, tag="tqm", name="t")
    for qt in range(4):
        nc.sync.dma_start(out=tq_m[32 * qt:32 * (qt + 1), :],
                          in_=queries_d[:, 128 * qt:128 * (qt + 1)])

    def tmp(tag):
        return cp.tile([128, 128], F32, tag=tag, name="t")

    # role r = j*4 + g (j = quad row [C1h C1l C2h C2l], g = group A B C' D');
    # j=4: roles 16-18 junk-zero, 19 = E
    for r in (0, 1, 2, 3, 8, 9, 10, 11):
        nc.vector.tensor_tensor(out=R(r), in0=R(r), in1=R(r + 4), op=OP.add)
    A1, B1, Cs1, Dr1, A2, B2, Cs2, Dr2 = (R(r) for r in (0, 1, 2, 3, 8, 9, 10, 11))

    blk = cp.tile([128, 128], U8, tag="blk", name="t")
    nc.vector.tensor_scalar(out=blk[:], in0=tq_m[:], scalar1=C1, scalar2=None,
                            op0=OP.is_ge)
    biasC1 = cp.tile([128, 1], F32, tag="biasC1", name="t")
    nc.vector.memset(biasC1[:], C1)
    biasC2 = cp.tile([128, 1], F32, tag="biasC2", name="t")
    nc.vector.memset(biasC2[:], C2)
    e1 = tmp("e1")
    nc.scalar.activation(e1[:], tq_m[:], ACTF.Exp, bias=biasC1[:], scale=-1.0)
    e2 = tmp("e2")
    nc.scalar.activation(e2[:], tq_m[:], ACTF.Exp, bias=biasC2[:], scale=-1.0)

    def sel(tag, on_true, on_false):
        o = tmp(tag)
        nc.vector.select(o, blk[:], on_true, on_false)
        return o

    esel = sel("esel", e2[:], e1[:])
    Asel = sel("Asel", A2, A1)
    Bsel = sel("Bsel", B2, B1)
    Csel = sel("Csel", Cs2, Cs1)
    Dsel = sel("Dsel", Dr2, Dr1)

    feat0 = tmp("feat0")
    nc.vector.tensor_tensor(out=feat0[:], in0=esel[:], in1=Asel[:], op=OP.mult)
    nc.vector.tensor_tensor(out=feat0[:], in0=feat0[:], in1=esel[:], op=OP.mult)
    feat1 = tmp("feat1")
    nc.vector.tensor_tensor(out=feat1[:], in0=esel[:], in1=Bsel[:], op=OP.mult)
    feat2 = tmp("feat2")
    nc.vector.tensor_tensor(out=feat2[:], in0=Dsel[:], in1=Csel[:], op=OP.subtract)
    nc.vector.tensor_tensor(out=feat2[:], in0=feat2[:], in1=esel[:], op=OP.mult)

    eff0 = tmp("eff0")
    nc.vector.tensor_scalar(out=eff0[:], in0=R(19), scalar1=-2.0,
                            scalar2=escol[:], op0=OP.mult, op1=OP.add)

    combo = tmp("combo")
    nc.vector.tensor_scalar(out=combo[:], in0=feat0[:], scalar1=wbbc[:, 0:1],
                            scalar2=None, op0=OP.mult)
    nc.vector.scalar_tensor_tensor(out=combo[:], in0=feat1[:], scalar=wbbc[:, 1:2],
                                   in1=combo[:], op0=OP.mult, op1=OP.add)
    nc.vector.scalar_tensor_tensor(out=combo[:], in0=feat2[:], scalar=negw2[:],
                                   in1=combo[:], op0=OP.mult, op1=OP.add)
    logits = tmp("logits")
    nc.vector.tensor_tensor(out=logits[:], in0=combo[:], in1=eff0[:], op=OP.mult)
    nc.vector.tensor_scalar(out=logits[:], in0=logits[:], scalar1=wbbc[:, 3:4],
                            scalar2=None, op0=OP.add)
    # zero the sentinel query (qt==1 rows, col 127)
    nc.vector.tensor_tensor(out=logits[:, 127:128], in0=logits[:, 127:128],
                            in1=padcol, op=OP.mult)

    hsum = cp.tile([128, 1], F32, tag="hsum", name="t")
    nc.vector.tensor_reduce(out=hsum[:], in_=logits[:], axis=AX.X, op=OP.add)
    expt = tmp("expt")
    intcol = cp.tile([128, 1], F32, tag="intcol", name="t")
    nc.scalar.activation(expt[:], logits[:], ACTF.Exp, accum_out=intcol[:])
    nc.vector.tensor_scalar(out=intcol[:], in0=intcol[:], scalar1=-RES,
                            scalar2=None, op0=OP.mult)
    qtmaski = cp.tile([128, 1], U8, tag="qtmaski", name="t")
    nc.vector.tensor_scalar(out=qtmaski[:], in0=qtmask, scalar1=0.5,
                            scalar2=None, op0=OP.is_ge)
    rowpart = cp.tile([128, 1], F32, tag="rowpart", name="t")
    nc.vector.select(rowpart[:], qtmaski[:], hsum[:], intcol[:])
    nc.sync.dma_start(out=out_d[:], in_=rowpart[:])

    for pool in (pp, gp, mp, qp, sp, cp):
        pool.release()


_NC_CACHE = []


def _get_nc():
    if not _NC_CACHE:
        _NC_CACHE.append(build_nc())
    return _NC_CACHE[0]


def make_inputs_for_core(times, states, base, weights, core):
    grid = np.arange(0.0, T_MAX, RES, dtype=np.float32)
    consts = np.ones((128, 2), np.float32)
    consts[64:128, 0] = 0.0   # qtmask: 0 for grid rows (qt 2,3 blocks)
    consts[32:64, 1] = 0.0    # sentinel-column mask: 0 for qt1 block
    sl = slice(core * S, (core + 1) * S)
    t = np.ascontiguousarray(times[sl]).astype(np.float32)
    st = np.ascontiguousarray(states[sl]).astype(np.int32)
    queries = np.concatenate(
        [t[:, 4, 1:256], np.full((S, 1), BIG, np.float32),
         np.tile(grid, (S, 1))], axis=1).astype(np.float32)
    queriesp = (queries - np.float32(TOL)).astype(np.float32)
    times1p = (t[:, 1, :] - np.float32(TOL)).astype(np.float32)
    tT = np.ascontiguousarray(t.transpose(1, 2, 0).reshape(5, 2, EH, S))
    sT = np.ascontiguousarray(st.transpose(1, 2, 0).reshape(5, 2, EH, S))
    return {
        "timesT": tT,
        "statesT": sT,
        "base": np.asarray(base, np.float32),
        "weights": np.asarray(weights, np.float32),
        "queries": queries,
        "queriesp": queriesp,
        "times1p": times1p,
        "onesbf": np.ones(S * Q, mybir.dt.np(BF16)),
        "consts": consts,
    }


def kernel(times, states, base, weights):
    from concourse.bass_utils import run_bass_kernel_spmd

    times = np.asarray(times, np.float32)
    states = np.asarray(states, np.int32)
    nc = _get_nc()
    in_maps = [make_inputs_for_core(times, states, base, weights, c)
               for c in range(NCORES)]
    res = run_bass_kernel_spmd(nc, in_maps, list(range(NCORES)))
    parts = np.stack([np.asarray(res.results[c]["out"]) for c in range(NCORES)])
    total = np.sum(parts.astype(np.float32), dtype=np.float32)
    return np.array([total], dtype=np.float32)


def run_traced(times, states, base, weights):
    """Profiled run; returns BassKernelResults (exec_time_ns etc.)."""
    from concourse.bass_utils import run_bass_kernel_spmd

    times = np.asarray(times, np.float32)
    states = np.asarray(states, np.int32)
    nc = _get_nc()
    in_maps = [make_inputs_for_core(times, states, base, weights, c)
               for c in range(NCORES)]
    res = run_bass_kernel_spmd(nc, in_maps, list(range(NCORES)), trace=True)
    return res


# revision 66
# speedup vs baseline: 1.3003x; 1.0268x over previous
"""Trainium2 Bass kernel for nn_Logic_Learning_Model (declarative logic-rule
point-process log-likelihood).

Algorithm (factorized; validated vs reference at ~4e-7 rel err in numpy):
For each sample, all features are masked weighted sums over per-predicate
event arrays evaluated at 512 query times (255 head-event times + 1 big
sentinel + 256 grid points):

  feat0(q) = e^{2(Ck-tq)} * sum_j [t1_j < tq-TOL] * g_j(Ck)
             g_j(Ck) = [s1_j==1] * e^{2(t1_j-Ck)} * What_j
             What_j  = e^{C2-t1_j} * sum_i [t0_i < t1_j-TOL][s0_i==1] e^{t0_i-C2}
  feat1(q) = e^{Ck-tq} * sum_j [t2_j < tq-TOL] * [s2_j==1] e^{t2_j-Ck}
  feat2(q) = e^{Ck-tq} * ( D'(q) - C'(q) ),  D' = sum [t3<=tq] v3,
             C' = sum [t3<tq-TOL] v3,  v3_j = [s3_j==0] e^{t3_j-Ck}
  sh[idx(q)] = sum_j [tq > th_j] * (sh_j - sh_{j-1,wrap}) + sh_255

Ck is a per-query-block shift (C1=38.4 for tq<38.4, C2=76.8 otherwise) to
keep every exponential inside fp32 range.  Masks are exact 0/1 bf16 tiles:
 - mA/mB/mC: ACT-engine Identity activation computes diff = fl(tq-TOL) - t
   via the fast per-partition bias path; DVE/Pool immediate-scalar compares
   produce the 0/1 masks (exact fp32 reference rounding).
 - mD/mE/mwt: PE outer-product diffs from exact bf16 triple-splits
   (hi+mid+lo == fp32 value), compared on Pool/DVE.
Weighted sums run on the PE as bf16 matmuls with Dekker-split (hi+lo)
weight vectors accumulating in fp32 PSUM; all five groups share one PSUM
bank per sample (rows 0-3 A, 32-35 B, 64-67 C', 96-99 D', 100 E).

Sharding: pure data parallel, 32 samples per core on 8 cores; each core
returns 128 per-(sample,query-tile) partial sums; host adds them up.
"""

import numpy as np

import concourse.bass as bass
import concourse.mybir as mybir
from concourse.tile import TileContext

F32 = mybir.dt.float32
BF16 = mybir.dt.bfloat16
I32 = mybir.dt.int32
U8 = mybir.dt.uint8

NCORES = 8
S = 32          # samples per core
E = 256         # events per predicate
EH = 128        # half (one partition tile)
Q = 512         # query count: 255 head + 1 big sentinel + 256 grid
T_MAX = 76.8
RES = 0.3
TOL = 0.1
C1 = 38.4
C2 = 76.8
BIG = 1000.0

AX = mybir.AxisListType
OP = mybir.AluOpType
ACTF = mybir.ActivationFunctionType


def bcast(ap, n=128):
    """0-stride partition broadcast view of a flat DRAM AP."""
    return bass.AP(ap.tensor, ap.offset, [[0, n]] + list(ap.ap))


def build_nc():
    from concourse.bacc import Bacc
    nc = Bacc(None, target_bir_lowering=False)
    timesT_d = nc.dram_tensor("timesT", [5, 2, EH, S], F32, kind="ExternalInput")
    statesT_d = nc.dram_tensor("statesT", [5, 2, EH, S], I32, kind="ExternalInput")
    base_d = nc.dram_tensor("base", [1], F32, kind="ExternalInput")
    weights_d = nc.dram_tensor("weights", [3], F32, kind="ExternalInput")
    # queries[s] = [th[1:256], BIG, grid];  queriesp = fl(queries - TOL)
    queries_d = nc.dram_tensor("queries", [S, Q], F32, kind="ExternalInput")
    queriesp_d = nc.dram_tensor("queriesp", [S, Q], F32, kind="ExternalInput")
    times1p_d = nc.dram_tensor("times1p", [S, E], F32, kind="ExternalInput")
    onesbf_d = nc.dram_tensor("onesbf", [S * Q], BF16, kind="ExternalInput")
    # consts[:, 0] = qtmask (1 for head rows), consts[:, 1] = pad column mask
    consts_d = nc.dram_tensor("consts", [128, 2], F32, kind="ExternalInput")
    out_d = nc.dram_tensor("out", [128], F32, kind="ExternalOutput")

    with TileContext(nc) as tc:
        _build(tc, nc, timesT_d, statesT_d, base_d, weights_d, queries_d,
               queriesp_d, times1p_d, onesbf_d, consts_d, out_d)
    nc.finalize()
    return nc


def _build(tc, nc, timesT_d, statesT_d, base_d, weights_d, queries_d,
           queriesp_d, times1p_d, onesbf_d, consts_d, out_d):
    cp = tc.alloc_tile_pool(name="const", bufs=1)
    sp = tc.alloc_tile_pool(name="scr", bufs=3)
    s1 = tc.alloc_tile_pool(name="scr1", bufs=1)
    pp = tc.alloc_tile_pool(name="psum", bufs=1, space="PSUM")

    # ---------------- phase 0: loads + batched prep ----------------
    T = {}
    ST = {}
    for a in range(5):
        for kt in range(2):
            t_t = cp.tile([EH, S], F32, tag=f"T{a}{kt}", name="t")
            nc.sync.dma_start(out=t_t[:], in_=timesT_d[a, kt])
            T[a, kt] = t_t
            s_t = cp.tile([EH, S], I32, tag=f"S{a}{kt}", name="t")
            nc.sync.dma_start(out=s_t[:], in_=statesT_d[a, kt])
            ST[a, kt] = s_t

    # base/weights broadcast columns
    wbbc = cp.tile([128, 4], F32, tag="wbbc", name="t")
    nc.vector.memset(wbbc[:], 0.0)
    nc.sync.dma_start(out=wbbc[:, 0:3], in_=bcast(weights_d[:]))
    nc.sync.dma_start(out=wbbc[:, 3:4], in_=bcast(base_d[:]))
    negw2 = cp.tile([128, 1], F32, tag="negw2", name="t")
    nc.vector.tensor_scalar(out=negw2[:], in0=wbbc[:, 2:3], scalar1=-1.0,
                            scalar2=None, op0=OP.mult)

    consts = cp.tile([128, 2], F32, tag="consts", name="t")
    nc.sync.dma_start(out=consts[:], in_=consts_d[:])
    qtmask = consts[:, 0:1]
    padcol = consts[:, 1:2]

    # sigmoid-mask bias columns: -SCALE*t.  ACT computes
    # sigmoid(SCALE*tq + bias) which saturates to exactly 0.0/1.0 in fp32,
    # fusing diff+compare into one ACT op (threshold noise ~eps*t, negligible).
    SCALE = 1.0e20
    negS = {}
    for a in (0, 1, 2, 3):
        for kt in range(2):
            m = cp.tile([EH, S], F32, tag=f"negS{a}{kt}", name="t")
            nc.vector.tensor_scalar(out=m[:], in0=T[a, kt][:], scalar1=-SCALE,
                                    scalar2=None, op0=OP.mult)
            negS[a, kt] = m

    # batched exponentials / state masks
    ew = {}
    sm = {}
    for kt in range(2):
        def _exp(tag, src, scale, off):
            arg = sp.tile([EH, S], F32, tag=f"arg{tag}{kt}", name="t")
            nc.vector.tensor_scalar(out=arg[:], in0=src[:], scalar1=scale,
                                    scalar2=off, op0=OP.mult, op1=OP.add)
            e_t = cp.tile([EH, S], F32, tag=f"e{tag}{kt}", name="t")
            nc.scalar.activation(e_t[:], arg[:], ACTF.Exp)
            return e_t

        ew["w0", kt] = _exp("w0", T[0, kt], 1.0, -C2)
        ew["c2t1", kt] = _exp("c2t1", T[1, kt], -1.0, C2)
        ew["g1", kt] = _exp("g1", T[1, kt], 2.0, -2.0 * C1)
        ew["g2", kt] = _exp("g2", T[1, kt], 2.0, -2.0 * C2)
        ew["v21", kt] = _exp("v21", T[2, kt], 1.0, -C1)
        ew["v22", kt] = _exp("v22", T[2, kt], 1.0, -C2)
        ew["v31", kt] = _exp("v31", T[3, kt], 1.0, -C1)
        ew["v32", kt] = _exp("v32", T[3, kt], 1.0, -C2)

        for a, val, tag in ((0, 1, "s0"), (1, 1, "s1"), (2, 1, "s2"), (3, 0, "s3")):
            m = cp.tile([EH, S], F32, tag=f"{tag}{kt}", name="t")
            nc.vector.tensor_scalar(out=m[:], in0=ST[a, kt][:], scalar1=val,
                                    scalar2=None, op0=OP.is_equal)
            sm[tag, kt] = m

        # [t3 <= C1]: zero v3C1 entries no C1-block query can select (keeps
        # PSUM partial sums small)
        m31 = cp.tile([EH, S], F32, tag=f"m31{kt}", name="t")
        nc.vector.tensor_scalar(out=m31[:], in0=T[3, kt][:], scalar1=C1,
                                scalar2=None, op0=OP.is_le)
        sm["m31", kt] = m31

    def dekker(dst, blk0, src32, tmp_tag, eng=None):
        """write bf16 (hi, lo) blocks of src32 [128, S] into dst block cols"""
        eng = eng or nc.vector
        hi = dst[:, blk0 * S:(blk0 + 1) * S]
        lo = dst[:, (blk0 + 1) * S:(blk0 + 2) * S]
        eng.tensor_copy(out=hi, in_=src32[:])
        tmp = sp.tile([EH, S], F32, tag=tmp_tag, name="t")
        eng.tensor_copy(out=tmp[:], in_=hi)
        eng.tensor_tensor(out=lo, in0=src32[:], in1=tmp[:], op=OP.subtract)

    # w0 pairs (feat0 inner sum weights)
    w0pair = {}
    for kt in range(2):
        w0 = sp.tile([EH, S], F32, tag=f"w0m{kt}", name="t")
        nc.vector.tensor_tensor(out=w0[:], in0=ew["w0", kt][:], in1=sm["s0", kt][:],
                                op=OP.mult)
        pair = cp.tile([EH, 2 * S], BF16, tag=f"w0pair{kt}", name="t")
        dekker(pair, 0, w0, f"w0tmp{kt}")
        w0pair[kt] = pair

    # v2 / v3 quads [128, 4*S]: cols 4s..4s+3 = [vC1h vC1l vC2h vC2l]
    vB = {}
    vC = {}
    for kt in range(2):
        q_b = cp.tile([EH, 4 * S], BF16, tag=f"vB{kt}", name="t")
        q_c = cp.tile([EH, 4 * S], BF16, tag=f"vC{kt}", name="t")
        for ver, (e2tag, e3tag) in enumerate((("v21", "v31"), ("v22", "v32"))):
            v2 = sp.tile([EH, S], F32, tag=f"v2m{kt}{ver}", name="t")
            nc.gpsimd.tensor_tensor(out=v2[:], in0=ew[e2tag, kt][:],
                                    in1=sm["s2", kt][:], op=OP.mult)
            dekker(q_b, 2 * ver, v2, f"dkb{kt}{ver}", eng=nc.gpsimd)
            v3 = sp.tile([EH, S], F32, tag=f"v3m{kt}{ver}", name="t")
            nc.vector.tensor_tensor(out=v3[:], in0=ew[e3tag, kt][:],
                                    in1=sm["s3", kt][:], op=OP.mult)
            if ver == 0:
                nc.vector.tensor_tensor(out=v3[:], in0=v3[:],
                                        in1=sm["m31", kt][:], op=OP.mult)
            dekker(q_c, 2 * ver, v3, f"dkc{kt}{ver}")
        vB[kt] = q_b
        vC[kt] = q_c

    # dsh (bf16): sh_j - sh_{j-1 (wrap)}; zero-padded [z z z z dsh]
    shm1 = {0: sp.tile([EH, S], I32, tag="shm10", name="t"),
            1: sp.tile([EH, S], I32, tag="shm11", name="t")}
    nc.vector.memset(shm1[0][:], 0)
    nc.vector.memset(shm1[1][:], 0)
    nc.sync.dma_start(out=shm1[0][1:128, :], in_=ST[4, 0][0:127, :])
    nc.sync.dma_start(out=shm1[0][0:1, :], in_=ST[4, 1][127:128, :])
    nc.sync.dma_start(out=shm1[1][1:128, :], in_=ST[4, 1][0:127, :])
    nc.sync.dma_start(out=shm1[1][0:1, :], in_=ST[4, 0][127:128, :])
    dsh = {}
    for kt in range(2):
        d = cp.tile([EH, 5 * S], BF16, tag=f"dsh{kt}", name="t")
        nc.vector.memset(d[:], 0.0)
        nc.vector.tensor_tensor(out=d[:, 4 * S:5 * S], in0=ST[4, kt][:],
                                in1=shm1[kt][:], op=OP.subtract)
        dsh[kt] = d

    # escol = 1 - 2*sh[255], per (sample,qt) partition column
    sh255row = sp.tile([1, S], I32, tag="sh255row", name="t")
    nc.sync.dma_start(out=sh255row[:], in_=ST[4, 1][127:128, :])
    esrow = cp.tile([1, S], F32, tag="esrow", name="t")
    nc.vector.tensor_scalar(out=esrow[:], in0=sh255row[:], scalar1=-2.0,
                            scalar2=1.0, op0=OP.mult, op1=OP.add)
    escol = cp.tile([128, 1], F32, tag="escol", name="t")
    nc.vector.memset(escol[:], 0.0)
    for qt in range(4):
        nc.sync.dma_start(out=escol[32 * qt:32 * (qt + 1), :], in_=esrow[0:1, :])

    # identity for PE transposes
    iot = sp.tile([128, 128], I32, tag="iot", name="t")
    nc.gpsimd.iota(iot[:], pattern=[[1, 128]], base=0, channel_multiplier=-1)
    ident = cp.tile([128, 128], F32, tag="ident", name="t")
    nc.vector.tensor_scalar(out=ident[:], in0=iot[:], scalar1=0,
                            scalar2=None, op0=OP.is_equal)

    # PSUM tiles (8 banks: acc0-3 | pd0-2 | psw); the transpose scratch
    # reuses pd tiles before any diff runs.  Each acc bank holds all four
    # accumulation groups at row offsets 0/32/64/96 (A, B, C', D'+E).
    psw = pp.tile([128, 4], F32, tag="psw", name="t")
    pdw_ps = [pp.tile([128, Q], F32, tag=f"pd{i}", name="t") for i in range(3)]
    acc_ps = [pp.tile([128, Q], F32, tag=f"acc{i}", name="t") for i in range(4)]

    # ---- triple-split helpers ----
    def split3(dst_list, src_ap, part, cols, eng):
        """exact fp32 = hi+mid+lo bf16 split; dst_list = 3 bf16 tiles"""
        hi, mid, lo = dst_list
        r1 = s1.tile([part, cols], F32, tag=f"s3r1_{part}_{cols}", name="t")
        r1f = s1.tile([part, cols], F32, tag=f"s3rf_{part}_{cols}", name="t")
        eng.tensor_copy(out=hi[:], in_=src_ap)
        eng.tensor_copy(out=r1f[:], in_=hi[:])
        eng.tensor_tensor(out=r1[:], in0=src_ap, in1=r1f[:], op=OP.subtract)
        eng.tensor_copy(out=mid[:], in_=r1[:])
        eng.tensor_copy(out=r1f[:], in_=mid[:])
        eng.tensor_tensor(out=r1[:], in0=r1[:], in1=r1f[:], op=OP.subtract)
        eng.tensor_copy(out=lo[:], in_=r1[:])

    # query rows + their splits: qrow [32, Q] (tq)
    qrow = s1.tile([S, Q], F32, tag="qrow", name="t")
    nc.sync.dma_start(out=qrow[:], in_=queries_d[:])

    qspl = [s1.tile([S, Q], BF16, tag=f"qspl{k}", name="t") for k in range(3)]
    split3(qspl, qrow[:], S, Q, nc.vector)

    # negated transposed event splits for the PE-diff stationaries:
    # t2+TOL (mB), t3 (mD), t4 (mE)
    ttspl = {}
    for i, (a, kt) in enumerate([(3, 0), (3, 1), (4, 0), (4, 1)]):
        ps = pdw_ps[i % 3]
        nc.tensor.transpose(ps[0:S, 0:128], T[a, kt][:], ident[:])
        tt = s1.tile([S, 128], F32, tag=f"tt{i%2}", name="t")
        nc.scalar.copy(tt[:], ps[0:S, 0:128])
        ntt = s1.tile([S, 128], F32, tag=f"ntt{i%2}", name="t")
        # mB compares against tq (not tq-TOL): fold TOL into the t2 side
        off = -TOL if a == 2 else 0.0
        nc.vector.tensor_scalar(out=ntt[:], in0=tt[:], scalar1=-1.0,
                                scalar2=off, op0=OP.mult, op1=OP.add)
        spl = [s1.tile([S, 128], BF16, tag=f"nts{a}{kt}{k}", name="t")
               for k in range(3)]
        split3(spl, ntt[:], S, 128, nc.vector)
        ttspl[a, kt] = spl

    # 6-partition stacks, free dim = sample-major:
    #   stationary [6, S*ncols]: rows 0-2 = -splits, rows 3-5 = 1
    #   rhs        [6, S*ncols]: rows 0-2 = 1, rows 3-5 = +query splits
    # per-sample operand = [0:6, s*ncols:(s+1)*ncols]  (base partition 0);
    # ones rows come from DRAM (engine memsets can't start at partition 3).
    def stack6(tagbase, ncols, split_src, neg_first):
        t = cp.tile([6, S * ncols], BF16, tag=tagbase, name="t")
        r0 = 0 if neg_first else 3
        o0 = 3 if neg_first else 0
        nc.gpsimd.dma_start(out=t[o0:o0 + 3, :],
                            in_=bcast(onesbf_d[0:S * ncols], 3))
        for k in range(3):
            nc.gpsimd.dma_start(out=t[r0 + k:r0 + k + 1, :],
                                in_=split_src[k][0:S, 0:ncols])
        return t

    statD = {}
    statE = {}
    for kt in range(2):
        statD[kt] = stack6(f"stD{kt}", 128, ttspl[3, kt], True)
        statE[kt] = stack6(f"stE{kt}", 128, ttspl[4, kt], True)
    rhsQ = stack6("rhQ", Q, qspl, False)
    s1.release()

    # per-sample pools allocated after s1's release so they reuse its space
    qp = tc.alloc_tile_pool(name="qbc", bufs=4)
    mp = tc.alloc_tile_pool(name="mask", bufs=4)
    gp = tc.alloc_tile_pool(name="stga", bufs=2)

    def stk(tile, s, ncols):
        return tile[0:6, s * ncols:(s + 1) * ncols]

    # ------------- phase 1: per-sample What (feat0 inner sums) -------------
    wst = cp.tile([128, 4 * S], F32, tag="wst", name="t")
    t1p_tiles = {}
    mwt_store = {}

    def dma_t1pbc(s):
        t = qp.tile([128, E], F32, tag="t1pbc", name="t")
        nc.sync.dma_start(out=t[:], in_=bcast(times1p_d[s, :]))
        t1p_tiles[s] = t

    dma_t1pbc(0)
    dma_t1pbc(1)
    for i in range(S + 1):
        if i < S:
            if i + 2 < S:
                dma_t1pbc(i + 2)
            mwts = []
            for ikt in range(2):
                mwt = mp.tile([128, E], BF16, tag=f"mwt{ikt}", name="t")
                nc.scalar.activation(mwt[:], t1p_tiles[i][:], ACTF.Sigmoid,
                                     bias=negS[0, ikt][:, i:i + 1], scale=SCALE)
                mwts.append(mwt)
            mwt_store[i] = mwts
        s = i - 1
        if 0 <= s:
            mwts = mwt_store.pop(s)
            for jkt in range(2):
                for ikt in range(2):
                    nc.tensor.matmul(psw[:, 2 * jkt:2 * jkt + 2],
                                     mwts[ikt][:, jkt * EH:(jkt + 1) * EH],
                                     w0pair[ikt][:, s::S][:, 0:2],
                                     start=(ikt == 0), stop=(ikt == 1))
            nc.vector.tensor_copy(out=wst[:, s::S][:, 0:4], in_=psw[:])

    # ------------- phase 2: batched g-vector assembly (feat0 weights) ------
    gA = {}
    for kt in range(2):
        wh = sp.tile([EH, S], F32, tag=f"wh{kt}", name="t")
        nc.vector.tensor_tensor(out=wh[:], in0=wst[:, 2 * kt * S:(2 * kt + 1) * S],
                                in1=wst[:, (2 * kt + 1) * S:(2 * kt + 2) * S],
                                op=OP.add)
        nc.vector.tensor_tensor(out=wh[:], in0=wh[:], in1=ew["c2t1", kt][:],
                                op=OP.mult)
        g_t = cp.tile([EH, 4 * S], BF16, tag=f"gA{kt}", name="t")
        for ver, etag in enumerate(("g1", "g2")):
            g32 = sp.tile([EH, S], F32, tag=f"g32{kt}{ver}", name="t")
            nc.vector.tensor_tensor(out=g32[:], in0=ew[etag, kt][:], in1=wh[:],
                                    op=OP.mult)
            nc.vector.tensor_tensor(out=g32[:], in0=g32[:], in1=sm["s1", kt][:],
                                    op=OP.mult)
            dekker(g_t, 2 * ver, g32, f"dkg{kt}{ver}")
        gA[kt] = g_t

    # ------------- phase 3: per-sample masks + weighted sums ---------------
    for t_ps in acc_ps:
        nc.vector.memset(t_ps[:], 0.0)
    stage2 = cp.tile([128, 20 * 128], F32, tag="stage2", name="t")
    pdi = 0

    tqp_tiles = {}
    mask_store = {}

    def dma_tqpbc(s):
        t = qp.tile([128, Q], F32, tag="tqpbc", name="t")
        nc.sync.dma_start(out=t[:], in_=bcast(queriesp_d[s, :]))
        tqp_tiles[s] = t

    def emit_masks(s):
        nonlocal pdi
        tqpbc = tqp_tiles.pop(s)
        masks = {}
        for kt in range(2):
            # mA/mB/mC: one fused ACT sigmoid-mask op each
            for grp, a in (("A", 1), ("B", 2), ("C", 3)):
                m = mp.tile([128, Q], BF16, tag=f"m{grp}{kt}", name="t")
                nc.scalar.activation(m[:], tqpbc[:], ACTF.Sigmoid,
                                     bias=negS[a, kt][:, s:s + 1], scale=SCALE)
                masks[grp, kt] = m
            # mD/mE: PE split-diffs (PSUM) + DVE compares (mE needs the
            # exact-0 self-compare at head queries)
            for grp, stat, op in (("D", statD[kt], OP.is_ge),
                                  ("E", statE[kt], OP.is_gt)):
                pd = pdw_ps[pdi % 3]
                pdi += 1
                nc.tensor.matmul(pd[:, 0:Q], stk(stat, s, 128),
                                 stk(rhsQ, s, Q), start=True, stop=True)
                m = mp.tile([128, Q], BF16, tag=f"m{grp}{kt}", name="t")
                nc.vector.tensor_scalar(out=m[:], in0=pd[:, 0:Q], scalar1=0.0,
                                        scalar2=None, op0=op)
                masks[grp, kt] = m
        mask_store[s] = masks

    def emit_consumers(s):
        # group g accumulates at acc rows 32g..32g+4 (one rotating bank)
        masks = mask_store.pop(s)
        acc = acc_ps[s % 4]
        for kt in range(2):
            st = (kt == 0)
            sp_ = (kt == 1)
            nc.tensor.matmul(acc[0:4, 0:Q], gA[kt][:, s::S][:, 0:4],
                             masks["A", kt][:], start=st, stop=sp_)
            nc.tensor.matmul(acc[32:36, 0:Q], vB[kt][:, s::S][:, 0:4],
                             masks["B", kt][:], start=st, stop=sp_)
            nc.tensor.matmul(acc[64:68, 0:Q], vC[kt][:, s::S][:, 0:4],
                             masks["C", kt][:], start=st, stop=sp_)
            # D'+E share rows 96-100 (E via the zero-padded dsh lhsT)
            nc.tensor.matmul(acc[96:101, 0:Q], dsh[kt][:, s::S][:, 0:5],
                             masks["E", kt][:], start=st, stop=sp_,
                             tile_position=(0, 96))
            nc.tensor.matmul(acc[96:100, 0:Q], vC[kt][:, s::S][:, 0:4],
                             masks["D", kt][:], start=False, stop=False,
                             skip_group_check=True, tile_position=(0, 96))

    def emit_drain(s):
        # drain acc row-group 32g -> stga col-block g (all starts aligned;
        # PSUM reads only on ACT/DVE), then scatter to stage2 rows (32qt+s)
        acc = acc_ps[s % 4]
        stga = gp.tile([5, 4 * Q], F32, tag="stga", name="t")
        nc.scalar.copy(stga[0:5, 0:Q], acc[0:5, 0:Q])
        for g in range(1, 4):
            nc.vector.tensor_copy(out=stga[0:5, g * Q:(g + 1) * Q],
                                  in_=acc[32 * g:32 * g + 5, 0:Q])
        srcv = stga[:].rearrange("p (g x) -> p g x", g=4)
        for qt in range(4):
            row = 32 * qt + s
            dst = stage2[row:row + 1, :].rearrange(
                "one (j g c) -> one j g c", j=5, g=4)
            eng = nc.sync if qt < 2 else nc.gpsimd
            eng.dma_start(out=dst[:, :, :, :],
                          in_=srcv[:, :, qt * 128:(qt + 1) * 128])

    # software pipeline: tqpbc prefetch (+2), masks (lag 0), consumers
    # (lag 1), drain+scatter (lag 2) — keeps ACT/PE/DVE streams decoupled
    dma_tqpbc(0)
    dma_tqpbc(1)
    dma_tqpbc(2)
    for i in range(S + 2):
        if i < S:
            if i + 3 < S:
                dma_tqpbc(i + 3)
            emit_masks(i)
        if 0 <= i - 1 < S:
            emit_consumers(i - 1)
        if 0 <= i - 2 < S:
            emit_drain(i - 2)

    # ------------- phase 4: batched post-processing ------------------------
    def R(r):
        return stage2[:, r * 128:(r + 1) * 128]

    # query matrix [128 (qt,s), 128]: tq_m[32qt+s, c] = queries[s, 128qt+c]
    tq_m = cp.tile([128, 128], F32, tag="tqm", name="t")
    for qt in range(4):
        nc.sync.dma_start(out=tq_m[32 * qt:32 * (qt + 1), :],
                          in_=queries_d[:, 128 * qt:128 * (qt + 1)])

    def tmp(tag):
        return cp.tile([128, 128], F32, tag=tag, name="t")

    # role r = j*4 + g (j = quad row [C1h C1l C2h C2l], g = group A B C' D');
    # j=4: roles 16-18 junk-zero, 19 = E
    for r in (0, 1, 2, 3, 8, 9, 10, 11):
        nc.vector.tensor_tensor(out=R(r), in0=R(r), in1=R(r + 4), op=OP.add)
    A1, B1, Cs1, Dr1, A2, B2, Cs2, Dr2 = (R(r) for r in (0, 1, 2, 3, 8, 9, 10, 11))

    blk = cp.tile([128, 128], U8, tag="blk", name="t")
    nc.vector.tensor_scalar(out=blk[:], in0=tq_m[:], scalar1=C1, scalar2=None,
                            op0=OP.is_ge)
    biasC1 = cp.tile([128, 1], F32, tag="biasC1", name="t")
    nc.vector.memset(biasC1[:], C1)
    biasC2 = cp.tile([128, 1], F32, tag="biasC2", name="t")
    nc.vector.memset(biasC2[:], C2)
    e1 = tmp("e1")
    nc.scalar.activation(e1[:], tq_m[:], ACTF.Exp, bias=biasC1[:], scale=-1.0)
    e2 = tmp("e2")
    nc.scalar.activation(e2[:], tq_m[:], ACTF.Exp, bias=biasC2[:], scale=-1.0)

    def sel(tag, on_true, on_false):
        o = tmp(tag)
        nc.vector.select(o, blk[:], on_true, on_false)
        return o

    esel = sel("esel", e2[:], e1[:])
    Asel = sel("Asel", A2, A1)
    Bsel = sel("Bsel", B2, B1)
    Csel = sel("Csel", Cs2, Cs1)
    Dsel = sel("Dsel", Dr2, Dr1)

    feat0 = tmp("feat0")
    nc.vector.tensor_tensor(out=feat0[:], in0=esel[:], in1=Asel[:], op=OP.mult)
    nc.vector.tensor_tensor(out=feat0[:], in0=feat0[:], in1=esel[:], op=OP.mult)
    feat1 = tmp("feat1")
    nc.vector.tensor_tensor(out=feat1[:], in0=esel[:], in1=Bsel[:], op=OP.mult)
    feat2 = tmp("feat2")
    nc.vector.tensor_tensor(out=feat2[:], in0=Dsel[:], in1=Csel[:], op=OP.subtract)
    nc.vector.tensor_tensor(out=feat2[:], in0=feat2[:], in1=esel[:], op=OP.mult)

    eff0 = tmp("eff0")
    nc.vector.tensor_scalar(out=eff0[:], in0=R(19), scalar1=-2.0,
                            scalar2=escol[:], op0=OP.mult, op1=OP.add)

    combo = tmp("combo")
    nc.vector.tensor_scalar(out=combo[:], in0=feat0[:], scalar1=wbbc[:, 0:1],
                            scalar2=None, op0=OP.mult)
    nc.vector.scalar_tensor_tensor(out=combo[:], in0=feat1[:], scalar=wbbc[:, 1:2],
                                   in1=combo[:], op0=OP.mult, op1=OP.add)
    nc.vector.scalar_tensor_tensor(out=combo[:], in0=feat2[:], scalar=negw2[:],
                                   in1=combo[:], op0=OP.mult, op1=OP.add)
    logits = tmp("logits")
    nc.vector.tensor_tensor(out=logits[:], in0=combo[:], in1=eff0[:], op=OP.mult)
    nc.vector.tensor_scalar(out=logits[:], in0=logits[:], scalar1=wbbc[:, 3:4],
                            scalar2=None, op0=OP.add)
    # zero the sentinel query (qt==1 rows, col 127)
    nc.vector.tensor_tensor(out=logits[:, 127:128], in0=logits[:, 127:128],
                            in1=padcol, op=OP.mult)

    hsum = cp.tile([128, 1], F32, tag="hsum", name="t")
    nc.vector.tensor_reduce(out=hsum[:], in_=logits[:], axis=AX.X, op=OP.add)
    expt = tmp("expt")
    intcol = cp.tile([128, 1], F32, tag="intcol", name="t")
    nc.scalar.activation(expt[:], logits[:], ACTF.Exp, accum_out=intcol[:])
    nc.vector.tensor_scalar(out=intcol[:], in0=intcol[:], scalar1=-RES,
                            scalar2=None, op0=OP.mult)
    qtmaski = cp.tile([128, 1], U8, tag="qtmaski", name="t")
    nc.vector.tensor_scalar(out=qtmaski[:], in0=qtmask, scalar1=0.5,
                            scalar2=None, op0=OP.is_ge)
    rowpart = cp.tile([128, 1], F32, tag="rowpart", name="t")
    nc.vector.select(rowpart[:], qtmaski[:], hsum[:], intcol[:])
    nc.sync.dma_start(out=out_d[:], in_=rowpart[:])

    for pool in (pp, gp, mp, qp, sp, cp):
        pool.release()


_NC_CACHE = []


def _get_nc():
    if not _NC_CACHE:
        _NC_CACHE.append(build_nc())
    return _NC_CACHE[0]


def make_inputs_for_core(times, states, base, weights, core):
    grid = np.arange(0.0, T_MAX, RES, dtype=np.float32)
    consts = np.ones((128, 2), np.float32)
    consts[64:128, 0] = 0.0   # qtmask: 0 for grid rows (qt 2,3 blocks)
    consts[32:64, 1] = 0.0    # sentinel-column mask: 0 for qt1 block
    sl = slice(core * S, (core + 1) * S)
    t = np.ascontiguousarray(times[sl]).astype(np.float32)
    st = np.ascontiguousarray(states[sl]).astype(np.int32)
    queries = np.concatenate(
        [t[:, 4, 1:256], np.full((S, 1), BIG, np.float32),
         np.tile(grid, (S, 1))], axis=1).astype(np.float32)
    queriesp = (queries - np.float32(TOL)).astype(np.float32)
    times1p = (t[:, 1, :] - np.float32(TOL)).astype(np.float32)
    tT = np.ascontiguousarray(t.transpose(1, 2, 0).reshape(5, 2, EH, S))
    sT = np.ascontiguousarray(st.transpose(1, 2, 0).reshape(5, 2, EH, S))
    return {
        "timesT": tT,
        "statesT": sT,
        "base": np.asarray(base, np.float32),
        "weights": np.asarray(weights, np.float32),
        "queries": queries,
        "queriesp": queriesp,
        "times1p": times1p,
        "onesbf": np.ones(S * Q, mybir.dt.np(BF16)),
        "consts": consts,
    }


def kernel(times, states, base, weights):
    from concourse.bass_utils import run_bass_kernel_spmd

    times = np.asarray(times, np.float32)
    states = np.asarray(states, np.int32)
    nc = _get_nc()
    in_maps = [make_inputs_for_core(times, states, base, weights, c)
               for c in range(NCORES)]
    res = run_bass_kernel_spmd(nc, in_maps, list(range(NCORES)))
    parts = np.stack([np.asarray(res.results[c]["out"]) for c in range(NCORES)])
    total = np.sum(parts.astype(np.float32), dtype=np.float32)
    return np.array([total], dtype=np.float32)


def run_traced(times, states, base, weights):
    """Profiled run; returns BassKernelResults (exec_time_ns etc.)."""
    from concourse.bass_utils import run_bass_kernel_spmd

    times = np.asarray(times, np.float32)
    states = np.asarray(states, np.int32)
    nc = _get_nc()
    in_maps = [make_inputs_for_core(times, states, base, weights, c)
               for c in range(NCORES)]
    res = run_bass_kernel_spmd(nc, in_maps, list(range(NCORES)), trace=True)
    return res
